# revision 1
# baseline (speedup 1.0000x reference)
"""Trainium2 Bass kernel for causal multi-head attention with RoPE.

Problem (hardcoded): B=2, S=2048, D=1024, H=16 heads, DK=64, double 1/sqrt(dk)
scaling, causal mask, RoPE (interleaved pairs).

Sharding over 8 cores: core c -> batch b=c//4, head-group g=c%4 (4 heads each).
Each core computes q/k/v projections for its heads from x[b], RoPE, causal
attention, and a partial output projection (its 256 columns of the contraction
with wo).  Host sums the 4 partials per batch.

Layout choices (all host-side prep, free at grade time):
  - xT  [D, S]   : x[b] transposed on host -> projections contract over d with
                   no on-chip transposes.
  - q/k in "T layout" [e_local, S] with a global evens/odds row permutation:
    tile A = even rope-components of all 4 heads (32 rows each), tile B = odds.
    RoPE is then 6 full-width tensor_tensor ops, no cross-partition copies.
  - scores computed directly transposed: scoresT[ks, qs] = k'^T q', K=32 per
    A/B part, accumulating pairs; heads (0,2) / (1,3) share row-groups 0/64 and
    32/96 for PE row-tiling concurrency.
  - causal block skipping: for ks-chunk c and qs-window only cols >= 128c are
    computed; the diagonal 128x128 block is masked by multiplying exp by a 0/1
    upper-triangular tile (exactly reproduces exp(x-1e9)==0).
  - v kept natural [s, e] and augmented with a ones column per head: the
    attn@v matmul (lhsT=v_aug) emits outT[dv,qs] plus the softmax denominator
    as row 64.  Division by the denominator is a reciprocal + partition
    broadcast (SBUF->SBUF DMA) + multiply, written straight into the rhs of
    the final projection.
"""

import os
import numpy as np

import concourse.bass as bass
import concourse.bacc as bacc
import concourse.mybir as mybir
import concourse.tile as tile
from concourse import bass_utils

F32 = mybir.dt.float32
BF16 = mybir.dt.bfloat16

B, S, D, H = 2, 2048, 1024, 16
DK = 64
NH = 4          # heads per core
EG = NH * DK    # 256 local e-dims per core
P = 128
NDC = D // P    # 8 d-chunks
NSC = S // P    # 16 s-chunks of 128
NSB = S // 512  # 4 s-blocks of 512
BIGNEG = 0.0    # masking done by 0/1 multiply after exp

_NC_CACHE = None


def _build_nc():
    nc = bacc.Bacc("TRN2", target_bir_lowering=False, debug=False, num_devices=8)

    xT = nc.dram_tensor("xT", [D, S], BF16, kind="ExternalInput")
    wqa = nc.dram_tensor("wqa", [D, P], BF16, kind="ExternalInput")
    wqb = nc.dram_tensor("wqb", [D, P], BF16, kind="ExternalInput")
    wka = nc.dram_tensor("wka", [D, P], BF16, kind="ExternalInput")
    wkb = nc.dram_tensor("wkb", [D, P], BF16, kind="ExternalInput")
    wvt = nc.dram_tensor("wvt", [D, EG], BF16, kind="ExternalInput")
    wot = nc.dram_tensor("wot", [EG, D], F32, kind="ExternalInput")
    cc = nc.dram_tensor("cc", [P, S], F32, kind="ExternalInput")
    ss = nc.dram_tensor("ss", [P, S], F32, kind="ExternalInput")
    tri = nc.dram_tensor("tri", [P, P], F32, kind="ExternalInput")
    fT = nc.dram_tensor("fT", [D, S], F32, kind="ExternalOutput")
    dbg = os.environ.get("BASS_KERNEL_DEBUG", "0") == "1"
    if dbg:
        dq = nc.dram_tensor("dq", [4, P, S], F32, kind="ExternalOutput")
        dv = nc.dram_tensor("dv", [P, NSC * NH * (DK + 1)], F32,
                            kind="ExternalOutput")
        do = nc.dram_tensor("do", [P, 2 * S], F32, kind="ExternalOutput")
        dop = nc.dram_tensor("dop", [NH, DK + 1, S], F32, kind="ExternalOutput")
        drc = nc.dram_tensor("drc", [NH, DK, S], F32, kind="ExternalOutput")

    with tile.TileContext(nc) as tc:
        const = tc.alloc_tile_pool(name="const", bufs=1)

        # ---- resident SBUF ----
        xT_sb = const.tile([P, NDC, S], BF16)
        nc.sync.dma_start(xT_sb, xT.ap().rearrange("(dc p) s -> p dc s", p=P))
        wqa_sb = const.tile([P, NDC, P], BF16)
        nc.sync.dma_start(wqa_sb, wqa.ap().rearrange("(dc p) e -> p dc e", p=P))
        wqb_sb = const.tile([P, NDC, P], BF16)
        nc.sync.dma_start(wqb_sb, wqb.ap().rearrange("(dc p) e -> p dc e", p=P))
        wka_sb = const.tile([P, NDC, P], BF16)
        nc.sync.dma_start(wka_sb, wka.ap().rearrange("(dc p) e -> p dc e", p=P))
        wkb_sb = const.tile([P, NDC, P], BF16)
        nc.sync.dma_start(wkb_sb, wkb.ap().rearrange("(dc p) e -> p dc e", p=P))
        wvt_sb = const.tile([P, NDC, EG], BF16)
        nc.sync.dma_start(wvt_sb, wvt.ap().rearrange("(dc p) e -> p dc e", p=P))
        wot_sb = const.tile([P, 2, D], F32)
        nc.sync.dma_start(wot_sb, wot.ap().rearrange("(dc p) e -> p dc e", p=P))
        cc_sb = const.tile([P, S], F32)
        nc.sync.dma_start(cc_sb, cc.ap())
        ss_sb = const.tile([P, S], F32)
        nc.sync.dma_start(ss_sb, ss.ap())
        tri_sb = const.tile([P, P], F32)
        nc.sync.dma_start(tri_sb, tri.ap())

        qa_sb = const.tile([P, S], F32)
        qb_sb = const.tile([P, S], F32)
        ka_sb = const.tile([P, S], F32)
        kb_sb = const.tile([P, S], F32)
        # v augmented with a ones column per head: [p, sc, h, 65]
        v_aug = const.tile([P, NSC, NH, DK + 1], F32)
        nc.vector.memset(v_aug[:, :, :, DK], 1.0)
        # rhs of final projection: rows = local d (head-major), 2 tiles of 128
        outT_sb = const.tile([P, 2, S], F32)
        onesE = const.tile([P, P], F32)
        nc.vector.memset(onesE, 1.0)

        # ---- phase 1: q/k projections + RoPE ----
        with tc.tile_pool(name="ppqk", bufs=2, space="PSUM") as ppqk, \
             tc.tile_pool(name="ropet", bufs=2) as ropet:
            for (wa_sb, wb_sb, oa_sb, ob_sb) in (
                (wqa_sb, wqb_sb, qa_sb, qb_sb),
                (wka_sb, wkb_sb, ka_sb, kb_sb),
            ):
                psA = ppqk.tile([P, S], F32, tag="pp")
                psB = ppqk.tile([P, S], F32, tag="pp")
                for dc in range(NDC):
                    for sb in range(NSB):
                        nc.tensor.matmul(
                            psA[:, 512 * sb:512 * sb + 512],
                            wa_sb[:, dc, :],
                            xT_sb[:, dc, 512 * sb:512 * sb + 512],
                            start=(dc == 0), stop=(dc == NDC - 1),
                        )
                for dc in range(NDC):
                    for sb in range(NSB):
                        nc.tensor.matmul(
                            psB[:, 512 * sb:512 * sb + 512],
                            wb_sb[:, dc, :],
                            xT_sb[:, dc, 512 * sb:512 * sb + 512],
                            start=(dc == 0), stop=(dc == NDC - 1),
                        )
                # RoPE: a' = a*cc - b*ss ; b' = a*ss + b*cc  (per 512-bank)
                for sb in range(NSB):
                    sl = slice(512 * sb, 512 * sb + 512)
                    t1 = ropet.tile([P, 512], F32, tag="t1")
                    t2 = ropet.tile([P, 512], F32, tag="t2")
                    nc.vector.scalar_tensor_tensor(t1, psA[:, sl], 1.0, cc_sb[:, sl], mybir.AluOpType.mult, mybir.AluOpType.mult)
                    nc.vector.scalar_tensor_tensor(t2, psB[:, sl], 1.0, ss_sb[:, sl], mybir.AluOpType.mult, mybir.AluOpType.mult)
                    nc.vector.scalar_tensor_tensor(oa_sb[:, sl], t1, 1.0, t2, mybir.AluOpType.mult, mybir.AluOpType.subtract)
                    t3 = ropet.tile([P, 512], F32, tag="t1")
                    t4 = ropet.tile([P, 512], F32, tag="t2")
                    nc.vector.scalar_tensor_tensor(t3, psA[:, sl], 1.0, ss_sb[:, sl], mybir.AluOpType.mult, mybir.AluOpType.mult)
                    nc.vector.scalar_tensor_tensor(t4, psB[:, sl], 1.0, cc_sb[:, sl], mybir.AluOpType.mult, mybir.AluOpType.mult)
                    nc.vector.scalar_tensor_tensor(ob_sb[:, sl], t3, 1.0, t4, mybir.AluOpType.mult, mybir.AluOpType.add)

        if dbg:
            for i, t in enumerate((qa_sb, qb_sb, ka_sb, kb_sb)):
                nc.sync.dma_start(dq.ap()[i], t)

        # ---- phase 1b: v projection (natural layout) ----
        with tc.tile_pool(name="ppv", bufs=3, space="PSUM") as ppv:
            for sc in range(NSC):
                pv = ppv.tile([P, EG], F32, tag="pv")
                for dc in range(NDC):
                    nc.tensor.matmul(
                        pv,
                        xT_sb[:, dc, P * sc:P * sc + P],
                        wvt_sb[:, dc, :],
                        start=(dc == 0), stop=(dc == NDC - 1),
                    )
                nc.scalar.copy(
                    v_aug[:, sc, :, 0:DK],
                    pv.rearrange("p (h e) -> p h e", h=NH),
                )

        if dbg:
            nc.sync.dma_start(dv.ap(), v_aug.rearrange("p a b c -> p (a b c)"))

        # ---- phase 2: attention ----
        inv64 = 1.0 / 64.0
        with tc.tile_pool(name="scps", bufs=2, space="PSUM") as scps_pool, \
             tc.tile_pool(name="outps", bufs=1, space="PSUM") as outps_pool, \
             tc.tile_pool(name="expsb", bufs=3) as expsb_pool, \
             tc.tile_pool(name="divp", bufs=4) as divp:
            for g in range(NSB):
                q0 = 512 * g
                outp = [
                    outps_pool.tile([DK + 1, 512], F32, name=f"outp{h}",
                                    tag=f"outp{h}")
                    for h in range(NH)
                ]
                nclast = 4 * g + 3
                for c in range(nclast + 1):
                    j0 = max(0, P * (c - 4 * g))      # first live col in window
                    w = 512 - j0
                    for pair in ((0, 2), (1, 3)):
                        sc_ps = scps_pool.tile([P, 2, 512], F32, tag="sc")
                        for h in pair:
                            ha, hb = 32 * h, 32 * h + 32
                            nc.tensor.matmul(
                                sc_ps[:, h // 2, j0:512],
                                ka_sb[ha:hb, P * c:P * c + P],
                                qa_sb[ha:hb, q0 + j0:q0 + 512],
                                start=True, stop=False,
                                tile_position=(32 * h, 0),
                            )
                            nc.tensor.matmul(
                                sc_ps[:, h // 2, j0:512],
                                kb_sb[ha:hb, P * c:P * c + P],
                                qb_sb[ha:hb, q0 + j0:q0 + 512],
                                start=False, stop=True,
                                tile_position=(32 * h, 0),
                            )
                        exp_sb = expsb_pool.tile([P, 2, 512], F32, tag="ex")
                        nc.scalar.activation(
                            exp_sb[:, :, j0:512], sc_ps[:, :, j0:512],
                            mybir.ActivationFunctionType.Exp,
                            scale=inv64,
                        )
                        if c >= 4 * g:  # diagonal block: zero out ks > qs
                            d0 = 128 * (c - 4 * g)
                            for hh in range(2):
                                nc.vector.scalar_tensor_tensor(
                                    exp_sb[:, hh, d0:d0 + P],
                                    exp_sb[:, hh, d0:d0 + P],
                                    1.0, tri_sb,
                                    mybir.AluOpType.mult,
                                    mybir.AluOpType.mult,
                                )
                        for h in pair:
                            nc.tensor.matmul(
                                outp[h][:, j0:512],
                                v_aug[:, c, h, :],
                                exp_sb[:, h // 2, j0:512],
                                start=(c == 0), stop=(c == nclast),
                                skip_group_check=True,
                            )
                # divide by denominators, write into final-projection rhs
                for h in range(NH):
                    if dbg:
                        dtmp = divp.tile([DK + 1, 512], F32, tag="dtmp")
                        nc.scalar.copy(dtmp, outp[h])
                        nc.sync.dma_start(
                            dop.ap()[h, :, q0:q0 + 512], dtmp)
                    # lane-aligned division chain (all ops stay on their
                    # own partitions; broadcast via K=1 matmul from p64)
                    denrow = divp.tile([P, 512], F32, tag="denrow")
                    nc.scalar.copy(denrow[DK:DK + 1, :], outp[h][DK:DK + 1, :])
                    reciprow = divp.tile([P, 512], F32, tag="reciprow")
                    nc.vector.reciprocal(reciprow[DK:DK + 1, :],
                                         denrow[DK:DK + 1, :])
                    bc_ps = scps_pool.tile([P, 2, 512], F32, tag="sc")
                    nc.tensor.matmul(bc_ps[:, 0, :], onesE[DK:DK + 1, :],
                                     reciprow[DK:DK + 1, :],
                                     start=True, stop=True,
                                     tile_position=(64, 0))
                    recipB = divp.tile([DK, 512], F32, tag="recipB")
                    nc.scalar.copy(recipB, bc_ps[0:DK, 0, :])
                    if dbg:
                        nc.sync.dma_start(
                            drc.ap()[h, :, q0:q0 + 512], recipB)
                    r0 = 64 * (h % 2)
                    nc.vector.scalar_tensor_tensor(
                        outT_sb[r0:r0 + DK, h // 2, q0:q0 + 512],
                        outp[h][0:DK, :],
                        1.0, recipB,
                        mybir.AluOpType.mult,
                        mybir.AluOpType.mult,
                    )

        if dbg:
            nc.sync.dma_start(do.ap(), outT_sb.rearrange("p a s -> p (a s)"))

        # ---- phase 3: final projection (partial over this core's 256 dims) ----
        with tc.tile_pool(name="fps", bufs=4, space="PSUM") as fps_pool, \
             tc.tile_pool(name="fsb", bufs=4) as fsb_pool:
            for ec in range(D // P):
                for sb in range(NSB):
                    fps = fps_pool.tile([P, 512], F32, tag="f")
                    for dc in range(2):
                        nc.tensor.matmul(
                            fps,
                            wot_sb[:, dc, P * ec:P * ec + P],
                            outT_sb[:, dc, 512 * sb:512 * sb + 512],
                            start=(dc == 0), stop=(dc == 1),
                        )
                    fsb = fsb_pool.tile([P, 512], F32, tag="fo")
                    nc.scalar.copy(fsb, fps)
                    nc.sync.dma_start(
                        fT.ap()[P * ec:P * ec + P, 512 * sb:512 * sb + 512],
                        fsb,
                    )
        const.release()
    nc.compile()
    return nc


def _host_inputs(x, freqs_cos, freqs_sin, wq, wk, wv, wo):
    """Build the 8 per-core input maps (all host-side numpy)."""
    bf = np.dtype("bfloat16") if hasattr(np, "bfloat16") else None
    import ml_dtypes
    bf16 = ml_dtypes.bfloat16

    cosT = np.ascontiguousarray(freqs_cos.T).astype(np.float32)  # [32, S]
    sinT = np.ascontiguousarray(freqs_sin.T).astype(np.float32)
    cc = np.tile(cosT, (4, 1))
    ss = np.tile(sinT, (4, 1))
    # tri[p, j] = 1 if p <= j else 0   (keep ks <= qs on the diagonal block)
    tri = np.triu(np.ones((P, P), dtype=np.float32))

    idxA = np.concatenate([64 * h + np.arange(0, 64, 2) for h in range(NH)])
    idxB = idxA + 1

    in_maps = []
    for core in range(8):
        b, g = core // 4, core % 4
        hs = slice(EG * g, EG * (g + 1))
        wq_g, wk_g = wq[hs], wk[hs]
        m = {
            "xT": np.ascontiguousarray(x[b].T).astype(bf16),
            "wqa": np.ascontiguousarray(wq_g[idxA].T).astype(bf16),
            "wqb": np.ascontiguousarray(wq_g[idxB].T).astype(bf16),
            "wka": np.ascontiguousarray(wk_g[idxA].T).astype(bf16),
            "wkb": np.ascontiguousarray(wk_g[idxB].T).astype(bf16),
            "wvt": np.ascontiguousarray(wv[hs].T).astype(bf16),
            "wot": np.ascontiguousarray(wo[:, hs].T).astype(np.float32),
            "cc": cc, "ss": ss, "tri": tri,
        }
        in_maps.append(m)
    return in_maps


def kernel(x, freqs_cos, freqs_sin, mask, wq, wk, wv, wo):
    global _NC_CACHE
    x = np.asarray(x, dtype=np.float32)
    freqs_cos = np.asarray(freqs_cos, dtype=np.float32)
    freqs_sin = np.asarray(freqs_sin, dtype=np.float32)
    wq = np.asarray(wq, dtype=np.float32)
    wk = np.asarray(wk, dtype=np.float32)
    wv = np.asarray(wv, dtype=np.float32)
    wo = np.asarray(wo, dtype=np.float32)

    if _NC_CACHE is None:
        _NC_CACHE = _build_nc()
    nc = _NC_CACHE

    in_maps = _host_inputs(x, freqs_cos, freqs_sin, wq, wk, wv, wo)
    trace = os.environ.get("BASS_KERNEL_TRACE", "0") == "1"
    res = bass_utils.run_bass_kernel_spmd(
        nc, in_maps, core_ids=list(range(8)), trace=trace,
    )
    if trace and res.exec_time_ns is not None:
        print(f"HW exec time: {res.exec_time_ns} ns")
        _tr = getattr(res, "instructions_and_trace", None)
        if _tr:
            print(f"trace: {_tr[1]}")

    out = np.zeros((B, S, D), dtype=np.float32)
    for core in range(8):
        b = core // 4
        out[b] += res.results[core]["fT"].T.astype(np.float32)
    return out



# revision 22
# speedup vs baseline: 1.7634x; 1.7634x over previous
"""Trainium2 Bass kernel for causal multi-head attention with RoPE.

Problem (hardcoded): B=2, S=2048, D=1024, H=16 heads, DK=64, double 1/sqrt(dk)
scaling, causal mask, RoPE (interleaved pairs).

Sharding over 8 cores: core c -> batch b=c//4, head-group g=c%4 (4 heads each).
Each core computes q/k/v projections for its heads from x[b], RoPE, causal
attention, and a partial output projection (its 256 columns of the contraction
with wo).  Host sums the 4 partials per batch.

Design notes (v2 — bf16 matmul operands, engine rebalance):
  - all matmul operands are bf16 (fp32 moving operands stream ~2x slower).
  - xT [D, S] host-transposed; q/k in "T layout" [e_local, S] with a global
    evens/odds row permutation: tile A = even rope components of all 4 heads
    (32 rows each), tile B = odds.  RoPE = 6 scalar_tensor_tensor per block.
  - scoresT[ks, qs] = k'^T q' with K=32 A/B accumulating pairs; heads at PE
    row bands 0/32/64/96 so 4 heads' matmuls run concurrently.
  - exp on the ACT engine (the true bottleneck: ~1 elem/lane/cycle @1.2GHz),
    psum->sbuf bf16, scale=1/64 fused.  Diagonal 128x128 blocks masked by a
    0/1 upper-tri multiply after exp (DVE, bf16 2x mode).
  - attn@v column-tiled: heads 2p/2p+1 at PE col bands (0,0)/(0,64), each
    streaming its own exp rhs -> out tile [128, 512] (dv rows packed 2 heads).
  - softmax denominators via 4 M=1 col-strip matmuls (ones lhsT) accumulating
    at psum partitions 0/32/64/96; reciprocal_approx_fast (DVE custom op);
    broadcast to 64-row bands with gpsimd partition_broadcast; one STT per
    out tile divides and writes the bf16 rhs of the final projection.
  - projections for q block g+1 and v chunks run interleaved with attention
    block g so the tensor engine fills ACT-bound gaps.
  - final projection at the end (psum->sbuf bf16 copies on any-engine), bf16
    DMA out; host sums the 4 partials per batch in fp32.
"""

import os
import numpy as np

import concourse.bass as bass
import concourse.bacc as bacc
import concourse.mybir as mybir
import concourse.tile as tile
from concourse import bass_utils

F32 = mybir.dt.float32
BF16 = mybir.dt.bfloat16
MULT = mybir.AluOpType.mult
ADD = mybir.AluOpType.add
SUB = mybir.AluOpType.subtract

B, S, D, H = 2, 2048, 1024, 16
DK = 64
NH = 4          # heads per core
EG = NH * DK    # 256 local e-dims per core
P = 128
NDC = D // P    # 8 d-chunks
NSC = S // P    # 16 s-chunks of 128
NSB = S // 512  # 4 s-blocks of 512

_NC_CACHE = None


def _build_nc():
    nc = bacc.Bacc("TRN2", target_bir_lowering=False, debug=False, num_devices=8)

    xT = nc.dram_tensor("xT", [D, S], BF16, kind="ExternalInput")
    wqa = nc.dram_tensor("wqa", [D, P], BF16, kind="ExternalInput")
    wqb = nc.dram_tensor("wqb", [D, P], BF16, kind="ExternalInput")
    wka = nc.dram_tensor("wka", [D, P], BF16, kind="ExternalInput")
    wkb = nc.dram_tensor("wkb", [D, P], BF16, kind="ExternalInput")
    wvt = nc.dram_tensor("wvt", [D, EG], BF16, kind="ExternalInput")
    wot = nc.dram_tensor("wot", [EG, D], BF16, kind="ExternalInput")
    cc = nc.dram_tensor("cc", [P, S], BF16, kind="ExternalInput")
    ss = nc.dram_tensor("ss", [P, S], BF16, kind="ExternalInput")
    tri2 = nc.dram_tensor("tri2", [P, 2 * P], BF16, kind="ExternalInput")
    sel = nc.dram_tensor("sel", [97, 2 * P], F32, kind="ExternalInput")
    fT = nc.dram_tensor("fT", [D, S], BF16, kind="ExternalOutput")
    dbg = os.environ.get("BASS_KERNEL_DEBUG", "0") == "1"
    if dbg:
        dq = nc.dram_tensor("dq", [4, P, S], F32, kind="ExternalOutput")
        dv = nc.dram_tensor("dv", [P, NSC * NH * DK], F32, kind="ExternalOutput")
        dden = nc.dram_tensor("dden", [NSB, P, 512], F32, kind="ExternalOutput")
        drecip = nc.dram_tensor("drecip", [NSB, P, 512], F32,
                                kind="ExternalOutput")
        dbc = nc.dram_tensor("dbc", [NSB, 2, P, 512], F32, kind="ExternalOutput")
        doutp = nc.dram_tensor("doutp", [NSB, 2, P, 512], F32,
                               kind="ExternalOutput")
        dexp = nc.dram_tensor("dexp", [2, P, 2, 512], F32, kind="ExternalOutput")
        dout = nc.dram_tensor("dout", [P, 2 * S], F32, kind="ExternalOutput")

    inv64 = 1.0 / 64.0

    with tile.TileContext(nc) as tc:
        const = tc.alloc_tile_pool(name="const", bufs=1)

        # ---- resident SBUF ----
        wka_sb = const.tile([P, NDC, P], BF16)
        nc.sync.dma_start(wka_sb, wka.ap().rearrange("(dc p) e -> p dc e", p=P))
        wkb_sb = const.tile([P, NDC, P], BF16)
        nc.sync.dma_start(wkb_sb, wkb.ap().rearrange("(dc p) e -> p dc e", p=P))
        xT_sb = const.tile([P, NDC, S], BF16)
        for dc in range(NDC):
            nc.sync.dma_start(
                xT_sb[:, dc, :], xT.ap()[P * dc:P * dc + P, :]
            )
        cc_sb = const.tile([P, S], BF16)
        nc.sync.dma_start(cc_sb, cc.ap())
        ss_sb = const.tile([P, S], BF16)
        nc.sync.dma_start(ss_sb, ss.ap())
        wqa_sb = const.tile([P, NDC, P], BF16)
        nc.sync.dma_start(wqa_sb, wqa.ap().rearrange("(dc p) e -> p dc e", p=P))
        wqb_sb = const.tile([P, NDC, P], BF16)
        nc.sync.dma_start(wqb_sb, wqb.ap().rearrange("(dc p) e -> p dc e", p=P))
        wvt_sb = const.tile([P, NDC, EG], BF16)
        nc.sync.dma_start(wvt_sb, wvt.ap().rearrange("(dc p) e -> p dc e", p=P))
        tri2_sb = const.tile([P, 2, P], BF16)
        nc.sync.dma_start(tri2_sb, tri2.ap().rearrange("p (t c) -> p t c", t=2))
        sel_sb = const.tile([97, 2, P], F32)
        nc.sync.dma_start(sel_sb, sel.ap().rearrange("p (t c) -> p t c", t=2))
        wot_sb = const.tile([P, 2, D], BF16)
        nc.sync.dma_start(wot_sb, wot.ap().rearrange("(dc p) e -> p dc e", p=P))

        qa_sb = const.tile([P, S], BF16)
        qb_sb = const.tile([P, S], BF16)
        ka_sb = const.tile([P, S], BF16)
        kb_sb = const.tile([P, S], BF16)
        v_sb = const.tile([P, NSC, NH, DK], BF16)
        outT_sb = const.tile([P, 2, S], BF16)
        ones_col = const.tile([P, 1], BF16)
        nc.vector.memset(ones_col, 1.0)

        def rope(pool, ps, col0, w, oa, ob):
            """ps: [P, 2, w] psum (A part slab 0, B slab 1) -> oa/ob bf16."""
            sl = slice(col0, col0 + w)
            t1 = pool.tile([P, 512], BF16, name="t1", tag="t1")
            t2 = pool.tile([P, 512], BF16, name="t2", tag="t2")
            nc.vector.scalar_tensor_tensor(
                t1[:, 0:w], ps[:, 0, :], 1.0, cc_sb[:, sl], MULT, MULT)
            nc.vector.scalar_tensor_tensor(
                t2[:, 0:w], ps[:, 1, :], 1.0, ss_sb[:, sl], MULT, MULT)
            nc.vector.scalar_tensor_tensor(
                oa[:, sl], t1[:, 0:w], 1.0, t2[:, 0:w], MULT, SUB)
            t3 = pool.tile([P, 512], BF16, name="t3", tag="t1")
            t4 = pool.tile([P, 512], BF16, name="t4", tag="t2")
            nc.vector.scalar_tensor_tensor(
                t3[:, 0:w], ps[:, 0, :], 1.0, ss_sb[:, sl], MULT, MULT)
            nc.vector.scalar_tensor_tensor(
                t4[:, 0:w], ps[:, 1, :], 1.0, cc_sb[:, sl], MULT, MULT)
            nc.vector.scalar_tensor_tensor(
                ob[:, sl], t3[:, 0:w], 1.0, t4[:, 0:w], MULT, ADD)

        def proj_block(pool, tag, wa, wb, col0, w, oa, ob, rope_pool, bufs=None):
            ps = pool.tile([P, 2, w], F32, name="ps", tag=tag, bufs=bufs)
            for i, wsel in enumerate((wa, wb)):
                for dc in range(NDC):
                    nc.tensor.matmul(
                        ps[:, i, :],
                        wsel[:, dc, :],
                        xT_sb[:, dc, col0:col0 + w],
                        start=(dc == 0), stop=(dc == NDC - 1),
                    )
            rope(rope_pool, ps, col0, w, oa, ob)

        def vchunk(pool, tag, sc, bufs=None):
            pv = pool.tile([P, 512], F32, name="pv", tag=tag, bufs=bufs)
            for dc in range(NDC):
                nc.tensor.matmul(
                    pv[:, 0:EG],
                    xT_sb[:, dc, P * sc:P * sc + P],
                    wvt_sb[:, dc, :],
                    start=(dc == 0), stop=(dc == NDC - 1),
                )
            nc.vector.tensor_copy(
                v_sb[:, sc, :, :],
                pv[:, 0:EG].rearrange("p (h e) -> p h e", h=NH))

        def vchunk2(pool, tag, sc0, bufs=None):
            # two s-chunks through one [P, 2, 256] tile so every allocation
            # of the shared tag covers the full slot extent (range-aware
            # WAR tracking on psum slot reuse needs matching extents).
            pv = pool.tile([P, 2, EG], F32, name="pv2", tag=tag, bufs=bufs)
            for k in range(2):
                for dc in range(NDC):
                    nc.tensor.matmul(
                        pv[:, k, :],
                        xT_sb[:, dc, P * (sc0 + k):P * (sc0 + k) + P],
                        wvt_sb[:, dc, :],
                        start=(dc == 0), stop=(dc == NDC - 1),
                    )
            for k in range(2):
                nc.vector.tensor_copy(
                    v_sb[:, sc0 + k, :, :],
                    pv[:, k, :].rearrange("p (h e) -> p h e", h=NH))

        # ---- phase 1: k (all), q block 0, v chunks 0-3 ----
        with tc.tile_pool(name="p1", bufs=3, space="PSUM") as p1, \
             tc.tile_pool(name="rp1", bufs=2) as rp1:
            # KQV_MODE: full = interleave q+v with attention; qonly/vonly
            # interleave just one kind; phased = everything upfront.
            _mode = os.environ.get("KQV_MODE", "phased")
            _il_q = _mode in ("full", "qonly")
            _il_v = _mode in ("full", "vonly")
            for sb in range(NSB):
                proj_block(p1, "kq", wka_sb, wkb_sb, 512 * sb, 512,
                           ka_sb, kb_sb, rp1)
            proj_block(p1, "kq", wqa_sb, wqb_sb, 0, 512, qa_sb, qb_sb, rp1)
            if not _il_q:
                for sb in range(1, NSB):
                    proj_block(p1, "kq", wqa_sb, wqb_sb, 512 * sb, 512,
                               qa_sb, qb_sb, rp1)
            for sc in range(4 if _il_v else NSC):
                vchunk(p1, "pv", sc, bufs=2)

        # ---- phase 2: attention, interleaved with q/v projections ----
        with tc.tile_pool(name="scp", bufs=2, space="PSUM") as scp, \
             tc.tile_pool(name="acc", bufs=1, space="PSUM") as acc, \
             tc.tile_pool(name="qvp", bufs=1, space="PSUM") as qvp, \
             tc.tile_pool(name="expp", bufs=2) as expp, \
             tc.tile_pool(name="rp2", bufs=2) as rp2, \
             tc.tile_pool(name="divp", bufs=2) as divp:
            for g in range(NSB):
                q0 = 512 * g
                nclast = 4 * g + 3
                outp = [
                    acc.tile([P, 512], F32, name=f"outp{t}", tag=f"outp{t}")
                    for t in range(2)
                ]
                den = acc.tile([P, 512], F32, name="den", tag="den")
                # garbage rows must stay finite: recip + selector matmul read
                # the full 0:97 partition range.
                nc.vector.memset(den, 1.0)
                for c in range(nclast + 1):
                    j0 = max(0, P * (c - 4 * g))
                    w = 512 - j0
                    for p in range(2):
                        sc = scp.tile([P, 2, 512], F32, name="sc", tag="sc")
                        for i in range(2):
                            h = 2 * p + i
                            ha = 32 * h
                            nc.tensor.matmul(
                                sc[:, i, j0:512],
                                ka_sb[ha:ha + 32, P * c:P * c + P],
                                qa_sb[ha:ha + 32, q0 + j0:q0 + 512],
                                start=True, stop=False,
                                tile_position=(ha, 0),
                            )
                            nc.tensor.matmul(
                                sc[:, i, j0:512],
                                kb_sb[ha:ha + 32, P * c:P * c + P],
                                qb_sb[ha:ha + 32, q0 + j0:q0 + 512],
                                start=False, stop=True,
                                tile_position=(ha, 0),
                            )
                        ex = expp.tile([P, 2, 512], BF16, name="ex", tag=f"ex{p}")
                        nc.scalar.activation(
                            ex[:, :, j0:512], sc[:, :, j0:512],
                            mybir.ActivationFunctionType.Exp,
                            scale=inv64,
                        )
                        if c >= 4 * g:  # diagonal block: zero ks > qs
                            nc.vector.scalar_tensor_tensor(
                                ex[:, :, j0:j0 + P],
                                ex[:, :, j0:j0 + P],
                                1.0, tri2_sb,
                                MULT, MULT,
                            )
                        if dbg and g == 1 and c == 0:
                            dext = expp.tile([P, 2, 512], F32, name="dext",
                                             tag=f"dext{p}")
                            nc.vector.tensor_copy(dext, ex)
                            nc.sync.dma_start(dexp.ap()[p], dext)
                        for i in range(2):
                            h = 2 * p + i
                            nc.tensor.matmul(
                                outp[p][64 * i:64 * i + 64, j0:512],
                                v_sb[:, c, h, :],
                                ex[:, i, j0:512],
                                start=(c == 0), stop=(c == nclast),
                                tile_position=(0, 64 * i),
                                skip_group_check=True,
                            )
                            nc.tensor.matmul(
                                den[32 * h:32 * h + 1, j0:512],
                                ones_col,
                                ex[:, i, j0:512],
                                start=(c == 0), stop=(c == nclast),
                                tile_position=(0, 32 * h),
                                skip_group_check=True,
                            )
                    if c == 0 and g < 3 and (_il_q or _il_v):
                        # interleave next block's q projection + the v chunks
                        # the NEXT block will need, at low priority so the
                        # tensor engine only runs them in ACT-bound gaps.
                        import contextlib
                        prio = (tc.high_priority(offset=-1_000_000)
                                if os.environ.get("KQV_PRIO", "1") == "1"
                                else contextlib.nullcontext())
                        with prio:
                            if _il_q:
                                for half in range(2):
                                    proj_block(qvp, "qv", wqa_sb, wqb_sb,
                                               512 * (g + 1) + 256 * half, 256,
                                               qa_sb, qb_sb, rp2)
                            if _il_v:
                                for sc2 in range(4 * g + 4, 4 * g + 8, 2):
                                    vchunk2(qvp, "qv", sc2)
                # division: recip of the 4 denominator rows, broadcast to
                # 64-row bands, multiply outp into the projection rhs.
                recip = divp.tile([P, 512], F32, name="recip", tag="recip")
                nc.vector.reciprocal_approx_fast(
                    recip[0:97, :], den[0:97, :])
                if dbg:
                    ddent = divp.tile([P, 512], F32, name="ddent", tag="ddent")
                    nc.vector.tensor_copy(ddent, den)
                    nc.sync.dma_start(dden.ap()[g], ddent)
                    nc.sync.dma_start(drecip.ap()[g], recip)
                for t in range(2):
                    bc_ps = acc.tile([P, 512], F32, name=f"bcps{t}", tag="den")
                    nc.tensor.matmul(
                        bc_ps, sel_sb[:, t, :], recip[0:97, :],
                        start=True, stop=True,
                    )
                    bc = divp.tile([P, 512], F32, name=f"bc{t}", tag=f"bc{t}")
                    nc.vector.tensor_copy(bc, bc_ps)
                    if dbg:
                        doutpt = divp.tile([P, 512], F32, name="doutpt",
                                           tag="ddent")
                        nc.vector.tensor_copy(doutpt, outp[t])
                        nc.sync.dma_start(doutp.ap()[g, t], doutpt)
                        nc.sync.dma_start(dbc.ap()[g, t], bc)
                    nc.vector.scalar_tensor_tensor(
                        outT_sb[:, t, q0:q0 + 512],
                        outp[t], 1.0, bc,
                        MULT, MULT,
                    )

        if dbg:
            for i, t in enumerate((qa_sb, qb_sb, ka_sb, kb_sb)):
                dqt = const.tile([P, S], F32, name=f"dq{i}")
                nc.vector.tensor_copy(dqt, t)
                nc.sync.dma_start(dq.ap()[i], dqt)
            dvt = const.tile([P, NSC * NH * DK], F32, name="dvt")
            nc.vector.tensor_copy(
                dvt, v_sb.rearrange("p a b c -> p (a b c)"))
            nc.sync.dma_start(dv.ap(), dvt)
            doutt = const.tile([P, 2 * S], F32, name="doutt")
            nc.vector.tensor_copy(
                doutt, outT_sb.rearrange("p a s -> p (a s)"))
            nc.sync.dma_start(dout.ap(), doutt)

        # ---- phase 3: final projection (partial over this core's 256 dims) ----
        with tc.tile_pool(name="fps", bufs=6, space="PSUM") as fps_pool, \
             tc.tile_pool(name="fsb", bufs=6) as fsb_pool:
            for ec in range(D // P):
                for sb in range(NSB):
                    fps = fps_pool.tile([P, 512], F32, name="fps", tag="f")
                    for dc in range(2):
                        nc.tensor.matmul(
                            fps,
                            wot_sb[:, dc, P * ec:P * ec + P],
                            outT_sb[:, dc, 512 * sb:512 * sb + 512],
                            start=(dc == 0), stop=(dc == 1),
                        )
                    fsb = fsb_pool.tile([P, 512], BF16, name="fsb", tag="fo")
                    nc.any.tensor_copy(fsb, fps)
                    nc.sync.dma_start(
                        fT.ap()[P * ec:P * ec + P, 512 * sb:512 * sb + 512],
                        fsb,
                    )
        const.release()
    nc.compile()
    return nc


def _host_inputs(x, freqs_cos, freqs_sin, wq, wk, wv, wo):
    """Build the 8 per-core input maps (all host-side numpy)."""
    import ml_dtypes
    bf16 = ml_dtypes.bfloat16

    cosT = np.ascontiguousarray(freqs_cos.T).astype(np.float32)  # [32, S]
    sinT = np.ascontiguousarray(freqs_sin.T).astype(np.float32)
    cc = np.tile(cosT, (4, 1)).astype(bf16)
    ss = np.tile(sinT, (4, 1)).astype(bf16)
    # tri[p, j] = 1 if p <= j else 0 (keep ks <= qs on the diagonal block)
    tri = np.triu(np.ones((P, P), dtype=np.float32))
    tri2 = np.concatenate([tri, tri], axis=1).astype(bf16)
    # selector for the denominator broadcast: out tile t rows 64i..64i+64
    # pick up recip row 32*(2t+i).
    sel = np.zeros((97, 2 * P), dtype=np.float32)
    for t in range(2):
        for i in range(2):
            sel[32 * (2 * t + i), P * t + 64 * i: P * t + 64 * i + 64] = 1.0

    idxA = np.concatenate([64 * h + np.arange(0, 64, 2) for h in range(NH)])
    idxB = idxA + 1

    in_maps = []
    for core in range(8):
        b, g = core // 4, core % 4
        hs = slice(EG * g, EG * (g + 1))
        wq_g, wk_g = wq[hs], wk[hs]
        m = {
            "xT": np.ascontiguousarray(x[b].T).astype(bf16),
            "wqa": np.ascontiguousarray(wq_g[idxA].T).astype(bf16),
            "wqb": np.ascontiguousarray(wq_g[idxB].T).astype(bf16),
            "wka": np.ascontiguousarray(wk_g[idxA].T).astype(bf16),
            "wkb": np.ascontiguousarray(wk_g[idxB].T).astype(bf16),
            "wvt": np.ascontiguousarray(wv[hs].T).astype(bf16),
            "wot": np.ascontiguousarray(wo[:, hs].T).astype(bf16),
            "cc": cc, "ss": ss, "tri2": tri2, "sel": sel,
        }
        in_maps.append(m)
    return in_maps


def kernel(x, freqs_cos, freqs_sin, mask, wq, wk, wv, wo):
    global _NC_CACHE
    x = np.asarray(x, dtype=np.float32)
    freqs_cos = np.asarray(freqs_cos, dtype=np.float32)
    freqs_sin = np.asarray(freqs_sin, dtype=np.float32)
    wq = np.asarray(wq, dtype=np.float32)
    wk = np.asarray(wk, dtype=np.float32)
    wv = np.asarray(wv, dtype=np.float32)
    wo = np.asarray(wo, dtype=np.float32)

    if _NC_CACHE is None:
        _NC_CACHE = _build_nc()
    nc = _NC_CACHE

    in_maps = _host_inputs(x, freqs_cos, freqs_sin, wq, wk, wv, wo)
    trace = os.environ.get("BASS_KERNEL_TRACE", "0") == "1"
    res = bass_utils.run_bass_kernel_spmd(
        nc, in_maps, core_ids=list(range(8)), trace=trace,
    )
    if trace and res.exec_time_ns is not None:
        print(f"HW exec time: {res.exec_time_ns} ns")
        _tr = getattr(res, "instructions_and_trace", None)
        if _tr:
            print(f"trace: {_tr[1]}")

    out = np.zeros((B, S, D), dtype=np.float32)
    for core in range(8):
        b = core // 4
        out[b] += res.results[core]["fT"].T.astype(np.float32)
    return out


# revision 23
# speedup vs baseline: 1.9847x; 1.1255x over previous
"""Trainium2 Bass kernel for causal multi-head attention with RoPE.

Problem (hardcoded): B=2, S=2048, D=1024, H=16 heads, DK=64, double 1/sqrt(dk)
scaling, causal mask, RoPE (interleaved pairs).

Sharding over 8 cores: core c -> batch b=c//4, head-group g=c%4 (4 heads each).
Each core computes q/k/v projections for its heads from x[b], RoPE, causal
attention, and a partial output projection (its 256 columns of the contraction
with wo).  Host sums the 4 partials per batch.

Design notes (v2 — bf16 matmul operands, engine rebalance):
  - all matmul operands are bf16 (fp32 moving operands stream ~2x slower).
  - xT [D, S] host-transposed; q/k in "T layout" [e_local, S] with a global
    evens/odds row permutation: tile A = even rope components of all 4 heads
    (32 rows each), tile B = odds.  RoPE = 6 scalar_tensor_tensor per block.
  - scoresT[ks, qs] = k'^T q' with K=32 A/B accumulating pairs; heads at PE
    row bands 0/32/64/96 so 4 heads' matmuls run concurrently.
  - exp on the ACT engine (the true bottleneck: ~1 elem/lane/cycle @1.2GHz),
    psum->sbuf bf16, scale=1/64 fused.  Diagonal 128x128 blocks masked by a
    0/1 upper-tri multiply after exp (DVE, bf16 2x mode).
  - attn@v column-tiled: heads 2p/2p+1 at PE col bands (0,0)/(0,64), each
    streaming its own exp rhs -> out tile [128, 512] (dv rows packed 2 heads).
  - softmax denominators via 4 M=1 col-strip matmuls (ones lhsT) accumulating
    at psum partitions 0/32/64/96; reciprocal_approx_fast (DVE custom op);
    broadcast to 64-row bands with gpsimd partition_broadcast; one STT per
    out tile divides and writes the bf16 rhs of the final projection.
  - projections for q block g+1 and v chunks run interleaved with attention
    block g so the tensor engine fills ACT-bound gaps.
  - final projection at the end (psum->sbuf bf16 copies on any-engine), bf16
    DMA out; host sums the 4 partials per batch in fp32.
"""

import os
import numpy as np

import concourse.bass as bass
import concourse.bacc as bacc
import concourse.mybir as mybir
import concourse.tile as tile
from concourse import bass_utils

F32 = mybir.dt.float32
BF16 = mybir.dt.bfloat16
MULT = mybir.AluOpType.mult
ADD = mybir.AluOpType.add
SUB = mybir.AluOpType.subtract

B, S, D, H = 2, 2048, 1024, 16
DK = 64
NH = 4          # heads per core
EG = NH * DK    # 256 local e-dims per core
P = 128
NDC = D // P    # 8 d-chunks
NSC = S // P    # 16 s-chunks of 128
NSB = S // 512  # 4 s-blocks of 512

_NC_CACHE = None


def _build_nc():
    nc = bacc.Bacc("TRN2", target_bir_lowering=False, debug=False, num_devices=8)

    xT = nc.dram_tensor("xT", [D, S], BF16, kind="ExternalInput")
    wqa = nc.dram_tensor("wqa", [D, P], BF16, kind="ExternalInput")
    wqb = nc.dram_tensor("wqb", [D, P], BF16, kind="ExternalInput")
    wka = nc.dram_tensor("wka", [D, P], BF16, kind="ExternalInput")
    wkb = nc.dram_tensor("wkb", [D, P], BF16, kind="ExternalInput")
    wvt = nc.dram_tensor("wvt", [D, EG], BF16, kind="ExternalInput")
    wot = nc.dram_tensor("wot", [EG, D], BF16, kind="ExternalInput")
    cc = nc.dram_tensor("cc", [P, S], BF16, kind="ExternalInput")
    ss = nc.dram_tensor("ss", [P, S], BF16, kind="ExternalInput")
    tri2 = nc.dram_tensor("tri2", [P, 2 * P], BF16, kind="ExternalInput")
    sel = nc.dram_tensor("sel", [97, 2 * P], F32, kind="ExternalInput")
    fT = nc.dram_tensor("fT", [D, S], BF16, kind="ExternalOutput")
    dbg = os.environ.get("BASS_KERNEL_DEBUG", "0") == "1"
    if dbg:
        dq = nc.dram_tensor("dq", [4, P, S], F32, kind="ExternalOutput")
        dv = nc.dram_tensor("dv", [P, NSC * NH * DK], F32, kind="ExternalOutput")
        dden = nc.dram_tensor("dden", [NSB, P, 512], F32, kind="ExternalOutput")
        drecip = nc.dram_tensor("drecip", [NSB, P, 512], F32,
                                kind="ExternalOutput")
        dbc = nc.dram_tensor("dbc", [NSB, 2, P, 512], F32, kind="ExternalOutput")
        doutp = nc.dram_tensor("doutp", [NSB, 2, P, 512], F32,
                               kind="ExternalOutput")
        dexp = nc.dram_tensor("dexp", [2, P, 2, 512], F32, kind="ExternalOutput")
        dout = nc.dram_tensor("dout", [P, 2 * S], F32, kind="ExternalOutput")

    inv64 = 1.0 / 64.0

    with tile.TileContext(nc) as tc:
        const = tc.alloc_tile_pool(name="const", bufs=1)

        # ---- resident SBUF ----
        wka_sb = const.tile([P, NDC, P], BF16)
        nc.sync.dma_start(wka_sb, wka.ap().rearrange("(dc p) e -> p dc e", p=P))
        wkb_sb = const.tile([P, NDC, P], BF16)
        nc.sync.dma_start(wkb_sb, wkb.ap().rearrange("(dc p) e -> p dc e", p=P))
        xT_sb = const.tile([P, NDC, S], BF16)
        for dc in range(NDC):
            nc.sync.dma_start(
                xT_sb[:, dc, :], xT.ap()[P * dc:P * dc + P, :]
            )
        cc_sb = const.tile([P, S], BF16)
        nc.sync.dma_start(cc_sb, cc.ap())
        ss_sb = const.tile([P, S], BF16)
        nc.sync.dma_start(ss_sb, ss.ap())
        wqa_sb = const.tile([P, NDC, P], BF16)
        nc.sync.dma_start(wqa_sb, wqa.ap().rearrange("(dc p) e -> p dc e", p=P))
        wqb_sb = const.tile([P, NDC, P], BF16)
        nc.sync.dma_start(wqb_sb, wqb.ap().rearrange("(dc p) e -> p dc e", p=P))
        wvt_sb = const.tile([P, NDC, EG], BF16)
        nc.sync.dma_start(wvt_sb, wvt.ap().rearrange("(dc p) e -> p dc e", p=P))
        tri2_sb = const.tile([P, 2, P], BF16)
        nc.sync.dma_start(tri2_sb, tri2.ap().rearrange("p (t c) -> p t c", t=2))
        sel_sb = const.tile([97, 2, P], F32)
        nc.sync.dma_start(sel_sb, sel.ap().rearrange("p (t c) -> p t c", t=2))
        wot_sb = const.tile([P, 2, D], BF16)
        nc.sync.dma_start(wot_sb, wot.ap().rearrange("(dc p) e -> p dc e", p=P))

        qa_sb = const.tile([P, S], BF16)
        qb_sb = const.tile([P, S], BF16)
        ka_sb = const.tile([P, S], BF16)
        kb_sb = const.tile([P, S], BF16)
        v_sb = const.tile([P, NSC, NH, DK], BF16)
        outT_sb = const.tile([P, 2, S], BF16)
        ones_col = const.tile([P, 1], BF16)
        nc.vector.memset(ones_col, 1.0)

        def rope(pool, ps, col0, w, oa, ob):
            """ps: [P, 2, w] psum (A part slab 0, B slab 1) -> oa/ob bf16."""
            sl = slice(col0, col0 + w)
            t1 = pool.tile([P, 512], BF16, name="t1", tag="t1")
            t2 = pool.tile([P, 512], BF16, name="t2", tag="t2")
            nc.vector.scalar_tensor_tensor(
                t1[:, 0:w], ps[:, 0, :], 1.0, cc_sb[:, sl], MULT, MULT)
            nc.vector.scalar_tensor_tensor(
                t2[:, 0:w], ps[:, 1, :], 1.0, ss_sb[:, sl], MULT, MULT)
            nc.vector.scalar_tensor_tensor(
                oa[:, sl], t1[:, 0:w], 1.0, t2[:, 0:w], MULT, SUB)
            t3 = pool.tile([P, 512], BF16, name="t3", tag="t1")
            t4 = pool.tile([P, 512], BF16, name="t4", tag="t2")
            nc.vector.scalar_tensor_tensor(
                t3[:, 0:w], ps[:, 0, :], 1.0, ss_sb[:, sl], MULT, MULT)
            nc.vector.scalar_tensor_tensor(
                t4[:, 0:w], ps[:, 1, :], 1.0, cc_sb[:, sl], MULT, MULT)
            nc.vector.scalar_tensor_tensor(
                ob[:, sl], t3[:, 0:w], 1.0, t4[:, 0:w], MULT, ADD)

        def proj_block(pool, tag, wa, wb, col0, w, oa, ob, rope_pool, bufs=None):
            ps = pool.tile([P, 2, w], F32, name="ps", tag=tag, bufs=bufs)
            for i, wsel in enumerate((wa, wb)):
                for dc in range(NDC):
                    nc.tensor.matmul(
                        ps[:, i, :],
                        wsel[:, dc, :],
                        xT_sb[:, dc, col0:col0 + w],
                        start=(dc == 0), stop=(dc == NDC - 1),
                    )
            rope(rope_pool, ps, col0, w, oa, ob)

        def vchunk(pool, tag, sc, bufs=None):
            pv = pool.tile([P, 512], F32, name="pv", tag=tag, bufs=bufs)
            for dc in range(NDC):
                nc.tensor.matmul(
                    pv[:, 0:EG],
                    xT_sb[:, dc, P * sc:P * sc + P],
                    wvt_sb[:, dc, :],
                    start=(dc == 0), stop=(dc == NDC - 1),
                )
            nc.vector.tensor_copy(
                v_sb[:, sc, :, :],
                pv[:, 0:EG].rearrange("p (h e) -> p h e", h=NH))

        def vchunk2(pool, tag, sc0, bufs=None):
            # two s-chunks through one [P, 2, 256] tile so every allocation
            # of the shared tag covers the full slot extent (range-aware
            # WAR tracking on psum slot reuse needs matching extents).
            pv = pool.tile([P, 2, EG], F32, name="pv2", tag=tag, bufs=bufs)
            for k in range(2):
                for dc in range(NDC):
                    nc.tensor.matmul(
                        pv[:, k, :],
                        xT_sb[:, dc, P * (sc0 + k):P * (sc0 + k) + P],
                        wvt_sb[:, dc, :],
                        start=(dc == 0), stop=(dc == NDC - 1),
                    )
            for k in range(2):
                nc.vector.tensor_copy(
                    v_sb[:, sc0 + k, :, :],
                    pv[:, k, :].rearrange("p (h e) -> p h e", h=NH))

        # ---- phase 1: k (all), q block 0, v chunks 0-3 ----
        with tc.tile_pool(name="p1", bufs=3, space="PSUM") as p1, \
             tc.tile_pool(name="rp1", bufs=2) as rp1:
            # KQV_MODE: full = interleave q+v with attention; qonly/vonly
            # interleave just one kind; phased = everything upfront.
            _mode = os.environ.get("KQV_MODE", "phased")
            _il_q = _mode in ("full", "qonly")
            _il_v = _mode in ("full", "vonly")
            for sb in range(NSB):
                proj_block(p1, "kq", wka_sb, wkb_sb, 512 * sb, 512,
                           ka_sb, kb_sb, rp1)
            proj_block(p1, "kq", wqa_sb, wqb_sb, 0, 512, qa_sb, qb_sb, rp1)
            if not _il_q:
                for sb in range(1, NSB):
                    proj_block(p1, "kq", wqa_sb, wqb_sb, 512 * sb, 512,
                               qa_sb, qb_sb, rp1)
            for sc in range(4 if _il_v else NSC):
                vchunk(p1, "pv", sc, bufs=2)

        # ---- phase 2: attention, interleaved with q/v projections ----
        with tc.tile_pool(name="scp", bufs=2, space="PSUM") as scp, \
             tc.tile_pool(name="acc", bufs=1, space="PSUM") as acc, \
             tc.tile_pool(name="qvp", bufs=1, space="PSUM") as qvp, \
             tc.tile_pool(name="expp", bufs=2) as expp, \
             tc.tile_pool(name="rp2", bufs=2) as rp2, \
             tc.tile_pool(name="divp", bufs=2) as divp:
            for g in range(NSB):
                q0 = 512 * g
                nclast = 4 * g + 3
                outp = [
                    acc.tile([P, 512], F32, name=f"outp{t}", tag=f"outp{t}")
                    for t in range(2)
                ]
                den = acc.tile([P, 512], F32, name="den", tag="den")
                # garbage rows must stay finite: recip + selector matmul read
                # the full 0:97 partition range.
                nc.vector.memset(den, 1.0)
                # two passes of 2 heads each: the sc pool then double-buffers
                # across CHUNKS, so exp(c) overlaps scores(c+1) and the PE
                # never drains (keeps HAM at full clock).
                for p in range(2):
                    for c in range(nclast + 1):
                        j0 = max(0, P * (c - 4 * g))
                        sc = scp.tile([P, 2, 512], F32, name="sc", tag="sc")
                        for i in range(2):
                            h = 2 * p + i
                            ha = 32 * h
                            nc.tensor.matmul(
                                sc[:, i, j0:512],
                                ka_sb[ha:ha + 32, P * c:P * c + P],
                                qa_sb[ha:ha + 32, q0 + j0:q0 + 512],
                                start=True, stop=False,
                                tile_position=(ha, 0),
                            )
                            nc.tensor.matmul(
                                sc[:, i, j0:512],
                                kb_sb[ha:ha + 32, P * c:P * c + P],
                                qb_sb[ha:ha + 32, q0 + j0:q0 + 512],
                                start=False, stop=True,
                                tile_position=(ha, 0),
                            )
                        ex = expp.tile([P, 2, 512], BF16, name="ex", tag=f"ex{p}")
                        nc.scalar.activation(
                            ex[:, :, j0:512], sc[:, :, j0:512],
                            mybir.ActivationFunctionType.Exp,
                            scale=inv64,
                        )
                        if c >= 4 * g:  # diagonal block: zero ks > qs
                            nc.vector.scalar_tensor_tensor(
                                ex[:, :, j0:j0 + P],
                                ex[:, :, j0:j0 + P],
                                1.0, tri2_sb,
                                MULT, MULT,
                            )
                        if dbg and g == 1 and c == 0:
                            dext = expp.tile([P, 2, 512], F32, name="dext",
                                             tag=f"dext{p}")
                            nc.vector.tensor_copy(dext, ex)
                            nc.sync.dma_start(dexp.ap()[p], dext)
                        for i in range(2):
                            h = 2 * p + i
                            nc.tensor.matmul(
                                outp[p][64 * i:64 * i + 64, j0:512],
                                v_sb[:, c, h, :],
                                ex[:, i, j0:512],
                                start=(c == 0), stop=(c == nclast),
                                tile_position=(0, 64 * i),
                                skip_group_check=True,
                            )
                            nc.tensor.matmul(
                                den[32 * h:32 * h + 1, j0:512],
                                ones_col,
                                ex[:, i, j0:512],
                                start=(c == 0), stop=(c == nclast),
                                tile_position=(0, 32 * h),
                                skip_group_check=True,
                            )
                    if p == 0 and g < 3 and (_il_q or _il_v):
                        # interleave next block's q projection + the v chunks
                        # the NEXT block will need, at low priority so the
                        # tensor engine only runs them in ACT-bound gaps.
                        import contextlib
                        prio = (tc.high_priority(offset=-1_000_000)
                                if os.environ.get("KQV_PRIO", "1") == "1"
                                else contextlib.nullcontext())
                        with prio:
                            if _il_q:
                                for half in range(2):
                                    proj_block(qvp, "qv", wqa_sb, wqb_sb,
                                               512 * (g + 1) + 256 * half, 256,
                                               qa_sb, qb_sb, rp2)
                            if _il_v:
                                for sc2 in range(4 * g + 4, 4 * g + 8, 2):
                                    vchunk2(qvp, "qv", sc2)
                # division: recip of the 4 denominator rows, broadcast to
                # 64-row bands, multiply outp into the projection rhs.
                recip = divp.tile([P, 512], F32, name="recip", tag="recip")
                nc.vector.reciprocal_approx_fast(
                    recip[0:97, :], den[0:97, :])
                if dbg:
                    ddent = divp.tile([P, 512], F32, name="ddent", tag="ddent")
                    nc.vector.tensor_copy(ddent, den)
                    nc.sync.dma_start(dden.ap()[g], ddent)
                    nc.sync.dma_start(drecip.ap()[g], recip)
                for t in range(2):
                    bc_ps = acc.tile([P, 512], F32, name=f"bcps{t}", tag="den")
                    nc.tensor.matmul(
                        bc_ps, sel_sb[:, t, :], recip[0:97, :],
                        start=True, stop=True,
                    )
                    bc = divp.tile([P, 512], F32, name=f"bc{t}", tag=f"bc{t}")
                    nc.vector.tensor_copy(bc, bc_ps)
                    if dbg:
                        doutpt = divp.tile([P, 512], F32, name="doutpt",
                                           tag="ddent")
                        nc.vector.tensor_copy(doutpt, outp[t])
                        nc.sync.dma_start(doutp.ap()[g, t], doutpt)
                        nc.sync.dma_start(dbc.ap()[g, t], bc)
                    nc.vector.scalar_tensor_tensor(
                        outT_sb[:, t, q0:q0 + 512],
                        outp[t], 1.0, bc,
                        MULT, MULT,
                    )

        if dbg:
            for i, t in enumerate((qa_sb, qb_sb, ka_sb, kb_sb)):
                dqt = const.tile([P, S], F32, name=f"dq{i}")
                nc.vector.tensor_copy(dqt, t)
                nc.sync.dma_start(dq.ap()[i], dqt)
            dvt = const.tile([P, NSC * NH * DK], F32, name="dvt")
            nc.vector.tensor_copy(
                dvt, v_sb.rearrange("p a b c -> p (a b c)"))
            nc.sync.dma_start(dv.ap(), dvt)
            doutt = const.tile([P, 2 * S], F32, name="doutt")
            nc.vector.tensor_copy(
                doutt, outT_sb.rearrange("p a s -> p (a s)"))
            nc.sync.dma_start(dout.ap(), doutt)

        # ---- phase 3: final projection (partial over this core's 256 dims) ----
        with tc.tile_pool(name="fps", bufs=6, space="PSUM") as fps_pool, \
             tc.tile_pool(name="fsb", bufs=6) as fsb_pool:
            for ec in range(D // P):
                for sb in range(NSB):
                    fps = fps_pool.tile([P, 512], F32, name="fps", tag="f")
                    for dc in range(2):
                        nc.tensor.matmul(
                            fps,
                            wot_sb[:, dc, P * ec:P * ec + P],
                            outT_sb[:, dc, 512 * sb:512 * sb + 512],
                            start=(dc == 0), stop=(dc == 1),
                        )
                    fsb = fsb_pool.tile([P, 512], BF16, name="fsb", tag="fo")
                    nc.any.tensor_copy(fsb, fps)
                    nc.sync.dma_start(
                        fT.ap()[P * ec:P * ec + P, 512 * sb:512 * sb + 512],
                        fsb,
                    )
        const.release()
    nc.compile()
    return nc


def _host_inputs(x, freqs_cos, freqs_sin, wq, wk, wv, wo):
    """Build the 8 per-core input maps (all host-side numpy)."""
    import ml_dtypes
    bf16 = ml_dtypes.bfloat16

    cosT = np.ascontiguousarray(freqs_cos.T).astype(np.float32)  # [32, S]
    sinT = np.ascontiguousarray(freqs_sin.T).astype(np.float32)
    cc = np.tile(cosT, (4, 1)).astype(bf16)
    ss = np.tile(sinT, (4, 1)).astype(bf16)
    # tri[p, j] = 1 if p <= j else 0 (keep ks <= qs on the diagonal block)
    tri = np.triu(np.ones((P, P), dtype=np.float32))
    tri2 = np.concatenate([tri, tri], axis=1).astype(bf16)
    # selector for the denominator broadcast: out tile t rows 64i..64i+64
    # pick up recip row 32*(2t+i).
    sel = np.zeros((97, 2 * P), dtype=np.float32)
    for t in range(2):
        for i in range(2):
            sel[32 * (2 * t + i), P * t + 64 * i: P * t + 64 * i + 64] = 1.0

    idxA = np.concatenate([64 * h + np.arange(0, 64, 2) for h in range(NH)])
    idxB = idxA + 1

    in_maps = []
    for core in range(8):
        b, g = core // 4, core % 4
        hs = slice(EG * g, EG * (g + 1))
        wq_g, wk_g = wq[hs], wk[hs]
        m = {
            "xT": np.ascontiguousarray(x[b].T).astype(bf16),
            "wqa": np.ascontiguousarray(wq_g[idxA].T).astype(bf16),
            "wqb": np.ascontiguousarray(wq_g[idxB].T).astype(bf16),
            "wka": np.ascontiguousarray(wk_g[idxA].T).astype(bf16),
            "wkb": np.ascontiguousarray(wk_g[idxB].T).astype(bf16),
            "wvt": np.ascontiguousarray(wv[hs].T).astype(bf16),
            "wot": np.ascontiguousarray(wo[:, hs].T).astype(bf16),
            "cc": cc, "ss": ss, "tri2": tri2, "sel": sel,
        }
        in_maps.append(m)
    return in_maps


def kernel(x, freqs_cos, freqs_sin, mask, wq, wk, wv, wo):
    global _NC_CACHE
    x = np.asarray(x, dtype=np.float32)
    freqs_cos = np.asarray(freqs_cos, dtype=np.float32)
    freqs_sin = np.asarray(freqs_sin, dtype=np.float32)
    wq = np.asarray(wq, dtype=np.float32)
    wk = np.asarray(wk, dtype=np.float32)
    wv = np.asarray(wv, dtype=np.float32)
    wo = np.asarray(wo, dtype=np.float32)

    if _NC_CACHE is None:
        _NC_CACHE = _build_nc()
    nc = _NC_CACHE

    in_maps = _host_inputs(x, freqs_cos, freqs_sin, wq, wk, wv, wo)
    trace = os.environ.get("BASS_KERNEL_TRACE", "0") == "1"
    res = bass_utils.run_bass_kernel_spmd(
        nc, in_maps, core_ids=list(range(8)), trace=trace,
    )
    if trace and res.exec_time_ns is not None:
        print(f"HW exec time: {res.exec_time_ns} ns")
        _tr = getattr(res, "instructions_and_trace", None)
        if _tr:
            print(f"trace: {_tr[1]}")

    out = np.zeros((B, S, D), dtype=np.float32)
    for core in range(8):
        b = core // 4
        out[b] += res.results[core]["fT"].T.astype(np.float32)
    return out


# revision 25
# speedup vs baseline: 2.1337x; 1.0751x over previous
"""Trainium2 Bass kernel for causal multi-head attention with RoPE.

Problem (hardcoded): B=2, S=2048, D=1024, H=16 heads, DK=64, double 1/sqrt(dk)
scaling, causal mask, RoPE (interleaved pairs).

Sharding over 8 cores: core c -> batch b=c//4, head-group g=c%4 (4 heads each).
Each core computes q/k/v projections for its heads from x[b], RoPE, causal
attention, and a partial output projection (its 256 columns of the contraction
with wo).  Host sums the 4 partials per batch.

Design notes (v2 — bf16 matmul operands, engine rebalance):
  - all matmul operands are bf16 (fp32 moving operands stream ~2x slower).
  - xT [D, S] host-transposed; q/k in "T layout" [e_local, S] with a global
    evens/odds row permutation: tile A = even rope components of all 4 heads
    (32 rows each), tile B = odds.  RoPE = 6 scalar_tensor_tensor per block.
  - scoresT[ks, qs] = k'^T q' with K=32 A/B accumulating pairs; heads at PE
    row bands 0/32/64/96 so 4 heads' matmuls run concurrently.
  - exp on the ACT engine (the true bottleneck: ~1 elem/lane/cycle @1.2GHz),
    psum->sbuf bf16, scale=1/64 fused.  Diagonal 128x128 blocks masked by a
    0/1 upper-tri multiply after exp (DVE, bf16 2x mode).
  - attn@v column-tiled: heads 2p/2p+1 at PE col bands (0,0)/(0,64), each
    streaming its own exp rhs -> out tile [128, 512] (dv rows packed 2 heads).
  - softmax denominators via 4 M=1 col-strip matmuls (ones lhsT) accumulating
    at psum partitions 0/32/64/96; reciprocal_approx_fast (DVE custom op);
    broadcast to 64-row bands with gpsimd partition_broadcast; one STT per
    out tile divides and writes the bf16 rhs of the final projection.
  - projections for q block g+1 and v chunks run interleaved with attention
    block g so the tensor engine fills ACT-bound gaps.
  - final projection at the end (psum->sbuf bf16 copies on any-engine), bf16
    DMA out; host sums the 4 partials per batch in fp32.
"""

import os
import numpy as np

import concourse.bass as bass
import concourse.bacc as bacc
import concourse.mybir as mybir
import concourse.tile as tile
from concourse import bass_utils

F32 = mybir.dt.float32
BF16 = mybir.dt.bfloat16
MULT = mybir.AluOpType.mult
ADD = mybir.AluOpType.add
SUB = mybir.AluOpType.subtract

B, S, D, H = 2, 2048, 1024, 16
DK = 64
NH = 4          # heads per core
EG = NH * DK    # 256 local e-dims per core
P = 128
NDC = D // P    # 8 d-chunks
NSC = S // P    # 16 s-chunks of 128
NSB = S // 512  # 4 s-blocks of 512

_NC_CACHE = None


def _build_nc():
    nc = bacc.Bacc("TRN2", target_bir_lowering=False, debug=False, num_devices=8)

    xT = nc.dram_tensor("xT", [D, S], BF16, kind="ExternalInput")
    wqa = nc.dram_tensor("wqa", [D, P], BF16, kind="ExternalInput")
    wqb = nc.dram_tensor("wqb", [D, P], BF16, kind="ExternalInput")
    wka = nc.dram_tensor("wka", [D, P], BF16, kind="ExternalInput")
    wkb = nc.dram_tensor("wkb", [D, P], BF16, kind="ExternalInput")
    wvt = nc.dram_tensor("wvt", [D, EG], BF16, kind="ExternalInput")
    wot = nc.dram_tensor("wot", [EG, D], BF16, kind="ExternalInput")
    cc = nc.dram_tensor("cc", [P, S], BF16, kind="ExternalInput")
    ss = nc.dram_tensor("ss", [P, S], BF16, kind="ExternalInput")
    tri2 = nc.dram_tensor("tri2", [P, 2 * P], BF16, kind="ExternalInput")
    sel = nc.dram_tensor("sel", [97, 2 * P], F32, kind="ExternalInput")
    fT = nc.dram_tensor("fT", [D, S], BF16, kind="ExternalOutput")
    dbg = os.environ.get("BASS_KERNEL_DEBUG", "0") == "1"
    if dbg:
        dq = nc.dram_tensor("dq", [4, P, S], F32, kind="ExternalOutput")
        dv = nc.dram_tensor("dv", [P, NSC * NH * DK], F32, kind="ExternalOutput")
        dden = nc.dram_tensor("dden", [NSB, P, 512], F32, kind="ExternalOutput")
        drecip = nc.dram_tensor("drecip", [NSB, P, 512], F32,
                                kind="ExternalOutput")
        dbc = nc.dram_tensor("dbc", [NSB, 2, P, 512], F32, kind="ExternalOutput")
        doutp = nc.dram_tensor("doutp", [NSB, 2, P, 512], F32,
                               kind="ExternalOutput")
        dexp = nc.dram_tensor("dexp", [2, P, 2, 512], F32, kind="ExternalOutput")
        dout = nc.dram_tensor("dout", [P, 2 * S], F32, kind="ExternalOutput")

    inv64 = 1.0 / 64.0

    with tile.TileContext(nc) as tc:
        const = tc.alloc_tile_pool(name="const", bufs=1)

        # ---- resident SBUF ----
        wka_sb = const.tile([P, NDC, P], BF16)
        nc.sync.dma_start(wka_sb, wka.ap().rearrange("(dc p) e -> p dc e", p=P))
        wkb_sb = const.tile([P, NDC, P], BF16)
        nc.sync.dma_start(wkb_sb, wkb.ap().rearrange("(dc p) e -> p dc e", p=P))
        xT_sb = const.tile([P, NDC, S], BF16)
        for dc in range(NDC):
            nc.sync.dma_start(
                xT_sb[:, dc, :], xT.ap()[P * dc:P * dc + P, :]
            )
        cc_sb = const.tile([P, S], BF16)
        nc.sync.dma_start(cc_sb, cc.ap())
        ss_sb = const.tile([P, S], BF16)
        nc.sync.dma_start(ss_sb, ss.ap())
        wqa_sb = const.tile([P, NDC, P], BF16)
        nc.sync.dma_start(wqa_sb, wqa.ap().rearrange("(dc p) e -> p dc e", p=P))
        wqb_sb = const.tile([P, NDC, P], BF16)
        nc.sync.dma_start(wqb_sb, wqb.ap().rearrange("(dc p) e -> p dc e", p=P))
        wvt_sb = const.tile([P, NDC, EG], BF16)
        nc.sync.dma_start(wvt_sb, wvt.ap().rearrange("(dc p) e -> p dc e", p=P))
        tri2_sb = const.tile([P, 2, P], BF16)
        nc.sync.dma_start(tri2_sb, tri2.ap().rearrange("p (t c) -> p t c", t=2))
        sel_sb = const.tile([97, 2, P], F32)
        nc.sync.dma_start(sel_sb, sel.ap().rearrange("p (t c) -> p t c", t=2))
        wot_sb = const.tile([P, 2, D], BF16)
        nc.sync.dma_start(wot_sb, wot.ap().rearrange("(dc p) e -> p dc e", p=P))

        qa_sb = const.tile([P, S], BF16)
        qb_sb = const.tile([P, S], BF16)
        ka_sb = const.tile([P, S], BF16)
        kb_sb = const.tile([P, S], BF16)
        v_sb = const.tile([P, NSC, NH, DK], BF16)
        outT_sb = const.tile([P, 2, S], BF16)
        ones_col = const.tile([P, 1], BF16)
        nc.vector.memset(ones_col, 1.0)

        def rope(pool, ps, col0, w, oa, ob):
            """ps: [P, 2, w] psum (A part slab 0, B slab 1) -> oa/ob bf16."""
            sl = slice(col0, col0 + w)
            t1 = pool.tile([P, 512], BF16, name="t1", tag="t1")
            t2 = pool.tile([P, 512], BF16, name="t2", tag="t2")
            nc.vector.scalar_tensor_tensor(
                t1[:, 0:w], ps[:, 0, :], 1.0, cc_sb[:, sl], MULT, MULT)
            nc.vector.scalar_tensor_tensor(
                t2[:, 0:w], ps[:, 1, :], 1.0, ss_sb[:, sl], MULT, MULT)
            nc.vector.scalar_tensor_tensor(
                oa[:, sl], t1[:, 0:w], 1.0, t2[:, 0:w], MULT, SUB)
            t3 = pool.tile([P, 512], BF16, name="t3", tag="t1")
            t4 = pool.tile([P, 512], BF16, name="t4", tag="t2")
            nc.vector.scalar_tensor_tensor(
                t3[:, 0:w], ps[:, 0, :], 1.0, ss_sb[:, sl], MULT, MULT)
            nc.vector.scalar_tensor_tensor(
                t4[:, 0:w], ps[:, 1, :], 1.0, cc_sb[:, sl], MULT, MULT)
            nc.vector.scalar_tensor_tensor(
                ob[:, sl], t3[:, 0:w], 1.0, t4[:, 0:w], MULT, ADD)

        def proj_block(pool, tag, wa, wb, col0, w, oa, ob, rope_pool, bufs=None):
            ps = pool.tile([P, 2, w], F32, name="ps", tag=tag, bufs=bufs)
            for i, wsel in enumerate((wa, wb)):
                for dc in range(NDC):
                    nc.tensor.matmul(
                        ps[:, i, :],
                        wsel[:, dc, :],
                        xT_sb[:, dc, col0:col0 + w],
                        start=(dc == 0), stop=(dc == NDC - 1),
                    )
            rope(rope_pool, ps, col0, w, oa, ob)

        def vchunk(pool, tag, sc, bufs=None):
            pv = pool.tile([P, 512], F32, name="pv", tag=tag, bufs=bufs)
            for dc in range(NDC):
                nc.tensor.matmul(
                    pv[:, 0:EG],
                    xT_sb[:, dc, P * sc:P * sc + P],
                    wvt_sb[:, dc, :],
                    start=(dc == 0), stop=(dc == NDC - 1),
                )
            nc.vector.tensor_copy(
                v_sb[:, sc, :, :],
                pv[:, 0:EG].rearrange("p (h e) -> p h e", h=NH))

        def vchunk2(pool, tag, sc0, bufs=None):
            # two s-chunks through one [P, 2, 256] tile so every allocation
            # of the shared tag covers the full slot extent (range-aware
            # WAR tracking on psum slot reuse needs matching extents).
            pv = pool.tile([P, 2, EG], F32, name="pv2", tag=tag, bufs=bufs)
            for k in range(2):
                for dc in range(NDC):
                    nc.tensor.matmul(
                        pv[:, k, :],
                        xT_sb[:, dc, P * (sc0 + k):P * (sc0 + k) + P],
                        wvt_sb[:, dc, :],
                        start=(dc == 0), stop=(dc == NDC - 1),
                    )
            for k in range(2):
                nc.vector.tensor_copy(
                    v_sb[:, sc0 + k, :, :],
                    pv[:, k, :].rearrange("p (h e) -> p h e", h=NH))

        # ---- phase 1: k (all), q block 0, v chunks 0-3 ----
        with tc.tile_pool(name="p1", bufs=3, space="PSUM") as p1, \
             tc.tile_pool(name="rp1", bufs=2) as rp1:
            # KQV_MODE: full = interleave q+v with attention; qonly/vonly
            # interleave just one kind; phased = everything upfront.
            _mode = os.environ.get("KQV_MODE", "phased")
            _il_q = _mode in ("full", "qonly")
            _il_v = _mode in ("full", "vonly")
            for sb in range(NSB):
                proj_block(p1, "kq", wka_sb, wkb_sb, 512 * sb, 512,
                           ka_sb, kb_sb, rp1)
            proj_block(p1, "kq", wqa_sb, wqb_sb, 0, 512, qa_sb, qb_sb, rp1)
            if not _il_q:
                for sb in range(1, NSB):
                    proj_block(p1, "kq", wqa_sb, wqb_sb, 512 * sb, 512,
                               qa_sb, qb_sb, rp1)
            for sc in range(4 if _il_v else NSC):
                vchunk(p1, "pv", sc, bufs=2)

        # ---- phase 2: attention, interleaved with q/v projections ----
        with tc.tile_pool(name="scp", bufs=2, space="PSUM") as scp, \
             tc.tile_pool(name="acc", bufs=1, space="PSUM") as acc, \
             tc.tile_pool(name="qvp", bufs=1, space="PSUM") as qvp, \
             tc.tile_pool(name="expp", bufs=2) as expp, \
             tc.tile_pool(name="rp2", bufs=2) as rp2, \
             tc.tile_pool(name="divp", bufs=2) as divp:
            for g in range(NSB):
                q0 = 512 * g
                nclast = 4 * g + 3
                outp = [
                    acc.tile([P, 512], F32, name=f"outp{t}", tag=f"outp{t}")
                    for t in range(2)
                ]
                den = acc.tile([P, 512], F32, name="den", tag="den")
                # garbage rows must stay finite: recip + selector matmul read
                # the full 0:97 partition range.
                nc.vector.memset(den, 1.0)
                # Two passes of 2 heads each (sc pool double-buffers across
                # chunks), with attn@v/den EMISSION deferred one chunk: the
                # tensor queue then goes [scores(c), av(c-1), scores(c+1),
                # av(c), ...] so by the time av(c) reaches the queue head its
                # exp(c) is long done — no per-chunk PE drain, HAM stays warm.
                def av_den(p, c, ex, j0):
                    for i in range(2):
                        h = 2 * p + i
                        nc.tensor.matmul(
                            outp[p][64 * i:64 * i + 64, j0:512],
                            v_sb[:, c, h, :],
                            ex[:, i, j0:512],
                            start=(c == 0), stop=(c == nclast),
                            tile_position=(0, 64 * i),
                            skip_group_check=True,
                        )
                        nc.tensor.matmul(
                            den[32 * h:32 * h + 1, j0:512],
                            ones_col,
                            ex[:, i, j0:512],
                            start=(c == 0), stop=(c == nclast),
                            tile_position=(0, 32 * h),
                            skip_group_check=True,
                        )

                prev = None
                for p in range(2):
                    for c in range(nclast + 1):
                        j0 = max(0, P * (c - 4 * g))
                        sc = scp.tile([P, 2, 512], F32, name="sc", tag="sc")
                        for i in range(2):
                            h = 2 * p + i
                            ha = 32 * h
                            nc.tensor.matmul(
                                sc[:, i, j0:512],
                                ka_sb[ha:ha + 32, P * c:P * c + P],
                                qa_sb[ha:ha + 32, q0 + j0:q0 + 512],
                                start=True, stop=False,
                                tile_position=(ha, 0),
                            )
                            nc.tensor.matmul(
                                sc[:, i, j0:512],
                                kb_sb[ha:ha + 32, P * c:P * c + P],
                                qb_sb[ha:ha + 32, q0 + j0:q0 + 512],
                                start=False, stop=True,
                                tile_position=(ha, 0),
                            )
                        ex = expp.tile([P, 2, 512], BF16, name="ex", tag=f"ex{p}")
                        nc.scalar.activation(
                            ex[:, :, j0:512], sc[:, :, j0:512],
                            mybir.ActivationFunctionType.Exp,
                            scale=inv64,
                        )
                        if c >= 4 * g:  # diagonal block: zero ks > qs
                            nc.vector.scalar_tensor_tensor(
                                ex[:, :, j0:j0 + P],
                                ex[:, :, j0:j0 + P],
                                1.0, tri2_sb,
                                MULT, MULT,
                            )
                        if dbg and g == 1 and c == 0:
                            dext = expp.tile([P, 2, 512], F32, name="dext",
                                             tag=f"dext{p}")
                            nc.vector.tensor_copy(dext, ex)
                            nc.sync.dma_start(dexp.ap()[p], dext)
                        if prev is not None:
                            av_den(*prev)
                        prev = (p, c, ex, j0)
                    if p == 0 and g < 3 and (_il_q or _il_v):
                        # interleave next block's q projection + the v chunks
                        # the NEXT block will need, at low priority so the
                        # tensor engine only runs them in ACT-bound gaps.
                        import contextlib
                        prio = (tc.high_priority(offset=-1_000_000)
                                if os.environ.get("KQV_PRIO", "1") == "1"
                                else contextlib.nullcontext())
                        with prio:
                            if _il_q:
                                for half in range(2):
                                    proj_block(qvp, "qv", wqa_sb, wqb_sb,
                                               512 * (g + 1) + 256 * half, 256,
                                               qa_sb, qb_sb, rp2)
                            if _il_v:
                                for sc2 in range(4 * g + 4, 4 * g + 8, 2):
                                    vchunk2(qvp, "qv", sc2)
                if prev is not None:
                    av_den(*prev)  # drain the deferred pipeline at block end
                # division: recip of the 4 denominator rows, broadcast to
                # 64-row bands, multiply outp into the projection rhs.
                recip = divp.tile([P, 512], F32, name="recip", tag="recip")
                nc.vector.reciprocal_approx_fast(
                    recip[0:97, :], den[0:97, :])
                if dbg:
                    ddent = divp.tile([P, 512], F32, name="ddent", tag="ddent")
                    nc.vector.tensor_copy(ddent, den)
                    nc.sync.dma_start(dden.ap()[g], ddent)
                    nc.sync.dma_start(drecip.ap()[g], recip)
                for t in range(2):
                    bc_ps = acc.tile([P, 512], F32, name=f"bcps{t}", tag="den")
                    nc.tensor.matmul(
                        bc_ps, sel_sb[:, t, :], recip[0:97, :],
                        start=True, stop=True,
                    )
                    bc = divp.tile([P, 512], F32, name=f"bc{t}", tag=f"bc{t}")
                    nc.vector.tensor_copy(bc, bc_ps)
                    if dbg:
                        doutpt = divp.tile([P, 512], F32, name="doutpt",
                                           tag="ddent")
                        nc.vector.tensor_copy(doutpt, outp[t])
                        nc.sync.dma_start(doutp.ap()[g, t], doutpt)
                        nc.sync.dma_start(dbc.ap()[g, t], bc)
                    nc.vector.scalar_tensor_tensor(
                        outT_sb[:, t, q0:q0 + 512],
                        outp[t], 1.0, bc,
                        MULT, MULT,
                    )

        if dbg:
            for i, t in enumerate((qa_sb, qb_sb, ka_sb, kb_sb)):
                dqt = const.tile([P, S], F32, name=f"dq{i}")
                nc.vector.tensor_copy(dqt, t)
                nc.sync.dma_start(dq.ap()[i], dqt)
            dvt = const.tile([P, NSC * NH * DK], F32, name="dvt")
            nc.vector.tensor_copy(
                dvt, v_sb.rearrange("p a b c -> p (a b c)"))
            nc.sync.dma_start(dv.ap(), dvt)
            doutt = const.tile([P, 2 * S], F32, name="doutt")
            nc.vector.tensor_copy(
                doutt, outT_sb.rearrange("p a s -> p (a s)"))
            nc.sync.dma_start(dout.ap(), doutt)

        # ---- phase 3: final projection (partial over this core's 256 dims) ----
        with tc.tile_pool(name="fps", bufs=6, space="PSUM") as fps_pool, \
             tc.tile_pool(name="fsb", bufs=6) as fsb_pool:
            for ec in range(D // P):
                for sb in range(NSB):
                    fps = fps_pool.tile([P, 512], F32, name="fps", tag="f")
                    for dc in range(2):
                        nc.tensor.matmul(
                            fps,
                            wot_sb[:, dc, P * ec:P * ec + P],
                            outT_sb[:, dc, 512 * sb:512 * sb + 512],
                            start=(dc == 0), stop=(dc == 1),
                        )
                    fsb = fsb_pool.tile([P, 512], BF16, name="fsb", tag="fo")
                    nc.any.tensor_copy(fsb, fps)
                    nc.sync.dma_start(
                        fT.ap()[P * ec:P * ec + P, 512 * sb:512 * sb + 512],
                        fsb,
                    )
        const.release()
    nc.compile()
    return nc


def _host_inputs(x, freqs_cos, freqs_sin, wq, wk, wv, wo):
    """Build the 8 per-core input maps (all host-side numpy)."""
    import ml_dtypes
    bf16 = ml_dtypes.bfloat16

    cosT = np.ascontiguousarray(freqs_cos.T).astype(np.float32)  # [32, S]
    sinT = np.ascontiguousarray(freqs_sin.T).astype(np.float32)
    cc = np.tile(cosT, (4, 1)).astype(bf16)
    ss = np.tile(sinT, (4, 1)).astype(bf16)
    # tri[p, j] = 1 if p <= j else 0 (keep ks <= qs on the diagonal block)
    tri = np.triu(np.ones((P, P), dtype=np.float32))
    tri2 = np.concatenate([tri, tri], axis=1).astype(bf16)
    # selector for the denominator broadcast: out tile t rows 64i..64i+64
    # pick up recip row 32*(2t+i).
    sel = np.zeros((97, 2 * P), dtype=np.float32)
    for t in range(2):
        for i in range(2):
            sel[32 * (2 * t + i), P * t + 64 * i: P * t + 64 * i + 64] = 1.0

    idxA = np.concatenate([64 * h + np.arange(0, 64, 2) for h in range(NH)])
    idxB = idxA + 1

    in_maps = []
    for core in range(8):
        b, g = core // 4, core % 4
        hs = slice(EG * g, EG * (g + 1))
        wq_g, wk_g = wq[hs], wk[hs]
        m = {
            "xT": np.ascontiguousarray(x[b].T).astype(bf16),
            "wqa": np.ascontiguousarray(wq_g[idxA].T).astype(bf16),
            "wqb": np.ascontiguousarray(wq_g[idxB].T).astype(bf16),
            "wka": np.ascontiguousarray(wk_g[idxA].T).astype(bf16),
            "wkb": np.ascontiguousarray(wk_g[idxB].T).astype(bf16),
            "wvt": np.ascontiguousarray(wv[hs].T).astype(bf16),
            "wot": np.ascontiguousarray(wo[:, hs].T).astype(bf16),
            "cc": cc, "ss": ss, "tri2": tri2, "sel": sel,
        }
        in_maps.append(m)
    return in_maps


def kernel(x, freqs_cos, freqs_sin, mask, wq, wk, wv, wo):
    global _NC_CACHE
    x = np.asarray(x, dtype=np.float32)
    freqs_cos = np.asarray(freqs_cos, dtype=np.float32)
    freqs_sin = np.asarray(freqs_sin, dtype=np.float32)
    wq = np.asarray(wq, dtype=np.float32)
    wk = np.asarray(wk, dtype=np.float32)
    wv = np.asarray(wv, dtype=np.float32)
    wo = np.asarray(wo, dtype=np.float32)

    if _NC_CACHE is None:
        _NC_CACHE = _build_nc()
    nc = _NC_CACHE

    in_maps = _host_inputs(x, freqs_cos, freqs_sin, wq, wk, wv, wo)
    trace = os.environ.get("BASS_KERNEL_TRACE", "0") == "1"
    res = bass_utils.run_bass_kernel_spmd(
        nc, in_maps, core_ids=list(range(8)), trace=trace,
    )
    if trace and res.exec_time_ns is not None:
        print(f"HW exec time: {res.exec_time_ns} ns")
        _tr = getattr(res, "instructions_and_trace", None)
        if _tr:
            print(f"trace: {_tr[1]}")

    out = np.zeros((B, S, D), dtype=np.float32)
    for core in range(8):
        b = core // 4
        out[b] += res.results[core]["fT"].T.astype(np.float32)
    return out


# revision 29
# speedup vs baseline: 2.2679x; 1.0629x over previous
"""Trainium2 Bass kernel for causal multi-head attention with RoPE.

Problem (hardcoded): B=2, S=2048, D=1024, H=16 heads, DK=64, double 1/sqrt(dk)
scaling, causal mask, RoPE (interleaved pairs).

Sharding over 8 cores: core c -> batch b=c//4, head-group g=c%4 (4 heads each).
Each core computes q/k/v projections for its heads from x[b], RoPE, causal
attention, and a partial output projection (its 256 columns of the contraction
with wo).  Host sums the 4 partials per batch.

Design notes (v2 — bf16 matmul operands, engine rebalance):
  - all matmul operands are bf16 (fp32 moving operands stream ~2x slower).
  - xT [D, S] host-transposed; q/k in "T layout" [e_local, S] with a global
    evens/odds row permutation: tile A = even rope components of all 4 heads
    (32 rows each), tile B = odds.  RoPE = 6 scalar_tensor_tensor per block.
  - scoresT[ks, qs] = k'^T q' with K=32 A/B accumulating pairs; heads at PE
    row bands 0/32/64/96 so 4 heads' matmuls run concurrently.
  - exp on the ACT engine (the true bottleneck: ~1 elem/lane/cycle @1.2GHz),
    psum->sbuf bf16, scale=1/64 fused.  Diagonal 128x128 blocks masked by a
    0/1 upper-tri multiply after exp (DVE, bf16 2x mode).
  - attn@v column-tiled: heads 2p/2p+1 at PE col bands (0,0)/(0,64), each
    streaming its own exp rhs -> out tile [128, 512] (dv rows packed 2 heads).
  - softmax denominators via 4 M=1 col-strip matmuls (ones lhsT) accumulating
    at psum partitions 0/32/64/96; reciprocal_approx_fast (DVE custom op);
    broadcast to 64-row bands with gpsimd partition_broadcast; one STT per
    out tile divides and writes the bf16 rhs of the final projection.
  - projections for q block g+1 and v chunks run interleaved with attention
    block g so the tensor engine fills ACT-bound gaps.
  - final projection at the end (psum->sbuf bf16 copies on any-engine), bf16
    DMA out; host sums the 4 partials per batch in fp32.
"""

import os
import numpy as np

import concourse.bass as bass
import concourse.bacc as bacc
import concourse.mybir as mybir
import concourse.tile as tile
from concourse import bass_utils

F32 = mybir.dt.float32
BF16 = mybir.dt.bfloat16
MULT = mybir.AluOpType.mult
ADD = mybir.AluOpType.add
SUB = mybir.AluOpType.subtract

B, S, D, H = 2, 2048, 1024, 16
DK = 64
NH = 4          # heads per core
EG = NH * DK    # 256 local e-dims per core
P = 128
NDC = D // P    # 8 d-chunks
NSC = S // P    # 16 s-chunks of 128
NSB = S // 512  # 4 s-blocks of 512

_NC_CACHE = None


def _build_nc():
    nc = bacc.Bacc("TRN2", target_bir_lowering=False, debug=False, num_devices=8)

    xT = nc.dram_tensor("xT", [D, S], BF16, kind="ExternalInput")
    wqa = nc.dram_tensor("wqa", [D, P], BF16, kind="ExternalInput")
    wqb = nc.dram_tensor("wqb", [D, P], BF16, kind="ExternalInput")
    wka = nc.dram_tensor("wka", [D, P], BF16, kind="ExternalInput")
    wkb = nc.dram_tensor("wkb", [D, P], BF16, kind="ExternalInput")
    wvt = nc.dram_tensor("wvt", [D, EG], BF16, kind="ExternalInput")
    wot = nc.dram_tensor("wot", [EG, D], BF16, kind="ExternalInput")
    cc = nc.dram_tensor("cc", [P, S], BF16, kind="ExternalInput")
    ss = nc.dram_tensor("ss", [P, S], BF16, kind="ExternalInput")
    tri2 = nc.dram_tensor("tri2", [P, 2 * P], BF16, kind="ExternalInput")
    sel = nc.dram_tensor("sel", [97, 2 * P], F32, kind="ExternalInput")
    fT = nc.dram_tensor("fT", [D, S], BF16, kind="ExternalOutput")
    dbg = os.environ.get("BASS_KERNEL_DEBUG", "0") == "1"
    if dbg:
        dq = nc.dram_tensor("dq", [4, P, S], F32, kind="ExternalOutput")
        dv = nc.dram_tensor("dv", [P, NSC * NH * DK], F32, kind="ExternalOutput")
        dden = nc.dram_tensor("dden", [NSB, P, 512], F32, kind="ExternalOutput")
        drecip = nc.dram_tensor("drecip", [NSB, P, 512], F32,
                                kind="ExternalOutput")
        dbc = nc.dram_tensor("dbc", [NSB, 2, P, 512], F32, kind="ExternalOutput")
        doutp = nc.dram_tensor("doutp", [NSB, 2, P, 512], F32,
                               kind="ExternalOutput")
        dexp = nc.dram_tensor("dexp", [2, P, 2, 512], F32, kind="ExternalOutput")
        dout = nc.dram_tensor("dout", [P, 2 * S], F32, kind="ExternalOutput")

    inv64 = 1.0 / 64.0

    with tile.TileContext(nc) as tc:
        const = tc.alloc_tile_pool(name="const", bufs=1)

        # ---- resident SBUF ----
        # DMA order tuned so the block-0 working set (k/q weights, cos/sin,
        # first 512 columns of xT) lands first and attention can start ~10us
        # in, while the rest streams behind.
        wka_sb = const.tile([P, NDC, P], BF16)
        nc.sync.dma_start(wka_sb, wka.ap().rearrange("(dc p) e -> p dc e", p=P))
        wkb_sb = const.tile([P, NDC, P], BF16)
        nc.sync.dma_start(wkb_sb, wkb.ap().rearrange("(dc p) e -> p dc e", p=P))
        wqa_sb = const.tile([P, NDC, P], BF16)
        nc.sync.dma_start(wqa_sb, wqa.ap().rearrange("(dc p) e -> p dc e", p=P))
        wqb_sb = const.tile([P, NDC, P], BF16)
        nc.sync.dma_start(wqb_sb, wqb.ap().rearrange("(dc p) e -> p dc e", p=P))
        cc_sb = const.tile([P, S], BF16)
        nc.sync.dma_start(cc_sb, cc.ap())
        ss_sb = const.tile([P, S], BF16)
        nc.sync.dma_start(ss_sb, ss.ap())
        wvt_sb = const.tile([P, NDC, EG], BF16)
        nc.sync.dma_start(wvt_sb, wvt.ap().rearrange("(dc p) e -> p dc e", p=P))
        tri2_sb = const.tile([P, 2, P], BF16)
        nc.sync.dma_start(tri2_sb, tri2.ap().rearrange("p (t c) -> p t c", t=2))
        sel_sb = const.tile([97, 2, P], F32)
        nc.sync.dma_start(sel_sb, sel.ap().rearrange("p (t c) -> p t c", t=2))
        xT_sb = const.tile([P, NDC, S], BF16)
        for cb in range(NSB):
            for dc in range(NDC):
                nc.sync.dma_start(
                    xT_sb[:, dc, 512 * cb:512 * cb + 512],
                    xT.ap()[P * dc:P * dc + P, 512 * cb:512 * cb + 512],
                )
        wot_sb = const.tile([P, 2, D], BF16)
        nc.sync.dma_start(wot_sb, wot.ap().rearrange("(dc p) e -> p dc e", p=P))

        qa_sb = const.tile([P, S], BF16)
        qb_sb = const.tile([P, S], BF16)
        ka_sb = const.tile([P, S], BF16)
        kb_sb = const.tile([P, S], BF16)
        v_sb = const.tile([P, NSC, NH, DK], BF16)
        outT_sb = const.tile([P, 2, S], BF16)
        ones_col = const.tile([P, 1], BF16)
        nc.vector.memset(ones_col, 1.0)

        def rope(pool, pc, col0, w, oa, ob):
            """pc: [P, 2, w] bf16 sbuf (A part slab 0, B slab 1) -> oa/ob."""
            sl = slice(col0, col0 + w)
            t1 = pool.tile([P, 512], BF16, name="t1", tag="t1")
            t2 = pool.tile([P, 512], BF16, name="t2", tag="t2")
            nc.vector.scalar_tensor_tensor(
                t1[:, 0:w], pc[:, 0, :], 1.0, cc_sb[:, sl], MULT, MULT)
            nc.vector.scalar_tensor_tensor(
                t2[:, 0:w], pc[:, 1, :], 1.0, ss_sb[:, sl], MULT, MULT)
            nc.vector.scalar_tensor_tensor(
                oa[:, sl], t1[:, 0:w], 1.0, t2[:, 0:w], MULT, SUB)
            t3 = pool.tile([P, 512], BF16, name="t3", tag="t1")
            t4 = pool.tile([P, 512], BF16, name="t4", tag="t2")
            nc.vector.scalar_tensor_tensor(
                t3[:, 0:w], pc[:, 0, :], 1.0, ss_sb[:, sl], MULT, MULT)
            nc.vector.scalar_tensor_tensor(
                t4[:, 0:w], pc[:, 1, :], 1.0, cc_sb[:, sl], MULT, MULT)
            nc.vector.scalar_tensor_tensor(
                ob[:, sl], t3[:, 0:w], 1.0, t4[:, 0:w], MULT, ADD)

        def proj_block(pool, tag, wa, wb, col0, w, oa, ob, rope_pool, bufs=None):
            ps = pool.tile([P, 2, w], F32, name="ps", tag=tag, bufs=bufs)
            for i, wsel in enumerate((wa, wb)):
                for dc in range(NDC):
                    nc.tensor.matmul(
                        ps[:, i, :],
                        wsel[:, dc, :],
                        xT_sb[:, dc, col0:col0 + w],
                        start=(dc == 0), stop=(dc == NDC - 1),
                    )
            # one CAST frees the psum bank immediately; RoPE then runs on
            # bf16 SBUF operands at DVE 2x rate.
            pc = rope_pool.tile([P, 2, 512], BF16, name="pc", tag="pc")
            nc.vector.tensor_copy(pc[:, :, 0:w], ps)
            rope(rope_pool, pc[:, :, 0:w], col0, w, oa, ob)

        def vchunk(pool, tag, sc, bufs=None):
            pv = pool.tile([P, 512], F32, name="pv", tag=tag, bufs=bufs)
            for dc in range(NDC):
                nc.tensor.matmul(
                    pv[:, 0:EG],
                    xT_sb[:, dc, P * sc:P * sc + P],
                    wvt_sb[:, dc, :],
                    start=(dc == 0), stop=(dc == NDC - 1),
                )
            nc.vector.tensor_copy(
                v_sb[:, sc, :, :],
                pv[:, 0:EG].rearrange("p (h e) -> p h e", h=NH))

        def vchunk2(pool, tag, sc0, bufs=None):
            # two s-chunks through one [P, 2, 256] tile so every allocation
            # of the shared tag covers the full slot extent (range-aware
            # WAR tracking on psum slot reuse needs matching extents).
            pv = pool.tile([P, 2, EG], F32, name="pv2", tag=tag, bufs=bufs)
            for k in range(2):
                for dc in range(NDC):
                    nc.tensor.matmul(
                        pv[:, k, :],
                        xT_sb[:, dc, P * (sc0 + k):P * (sc0 + k) + P],
                        wvt_sb[:, dc, :],
                        start=(dc == 0), stop=(dc == NDC - 1),
                    )
            for k in range(2):
                nc.vector.tensor_copy(
                    v_sb[:, sc0 + k, :, :],
                    pv[:, k, :].rearrange("p (h e) -> p h e", h=NH))

        # ---- phase 1: k (all), q block 0, v chunks 0-3 ----
        with tc.tile_pool(name="p1", bufs=3, space="PSUM") as p1, \
             tc.tile_pool(name="rp1", bufs=2) as rp1:
            # KQV_MODE: full = interleave k/q/v projections with attention;
            # phased = everything upfront.
            _mode = os.environ.get("KQV_MODE", "full")
            _il = _mode == "full"
            proj_block(p1, "kq", wka_sb, wkb_sb, 0, 512, ka_sb, kb_sb, rp1)
            proj_block(p1, "kq", wqa_sb, wqb_sb, 0, 512, qa_sb, qb_sb, rp1)
            if not _il:
                for sb in range(1, NSB):
                    proj_block(p1, "kq", wka_sb, wkb_sb, 512 * sb, 512,
                               ka_sb, kb_sb, rp1)
                for sb in range(1, NSB):
                    proj_block(p1, "kq", wqa_sb, wqb_sb, 512 * sb, 512,
                               qa_sb, qb_sb, rp1)
            for sc in range(4 if _il else NSC):
                vchunk(p1, "pv", sc, bufs=2)

        # ---- phase 2: attention, interleaved with q/v projections ----
        with tc.tile_pool(name="scp", bufs=2, space="PSUM") as scp, \
             tc.tile_pool(name="acc", bufs=1, space="PSUM") as acc, \
             tc.tile_pool(name="qvp", bufs=1, space="PSUM") as qvp, \
             tc.tile_pool(name="expp", bufs=2) as expp, \
             tc.tile_pool(name="rp2", bufs=2) as rp2, \
             tc.tile_pool(name="divp", bufs=2) as divp:
            for g in range(NSB):
                q0 = 512 * g
                nclast = 4 * g + 3
                outp = [
                    acc.tile([P, 512], F32, name=f"outp{t}", tag=f"outp{t}")
                    for t in range(2)
                ]
                den = acc.tile([P, 512], F32, name="den", tag="den")
                # garbage rows must stay finite: recip + selector matmul read
                # the full 0:97 partition range.
                nc.vector.memset(den, 1.0)
                # Two passes of 2 heads each (sc pool double-buffers across
                # chunks), with attn@v/den EMISSION deferred one chunk: the
                # tensor queue then goes [scores(c), av(c-1), scores(c+1),
                # av(c), ...] so by the time av(c) reaches the queue head its
                # exp(c) is long done — no per-chunk PE drain, HAM stays warm.
                def av_den(p, c, ex, j0):
                    for i in range(2):
                        h = 2 * p + i
                        nc.tensor.matmul(
                            outp[p][64 * i:64 * i + 64, j0:512],
                            v_sb[:, c, h, :],
                            ex[:, i, j0:512],
                            start=(c == 0), stop=(c == nclast),
                            tile_position=(0, 64 * i),
                            skip_group_check=True,
                        )
                        nc.tensor.matmul(
                            den[32 * h:32 * h + 1, j0:512],
                            ones_col,
                            ex[:, i, j0:512],
                            start=(c == 0), stop=(c == nclast),
                            tile_position=(0, 32 * h),
                            skip_group_check=True,
                        )

                prev = None
                for p in range(2):
                    for c in range(nclast + 1):
                        j0 = max(0, P * (c - 4 * g))
                        sc = scp.tile([P, 2, 512], F32, name="sc", tag="sc")
                        for i in range(2):
                            h = 2 * p + i
                            ha = 32 * h
                            nc.tensor.matmul(
                                sc[:, i, j0:512],
                                ka_sb[ha:ha + 32, P * c:P * c + P],
                                qa_sb[ha:ha + 32, q0 + j0:q0 + 512],
                                start=True, stop=False,
                                tile_position=(ha, 0),
                            )
                            nc.tensor.matmul(
                                sc[:, i, j0:512],
                                kb_sb[ha:ha + 32, P * c:P * c + P],
                                qb_sb[ha:ha + 32, q0 + j0:q0 + 512],
                                start=False, stop=True,
                                tile_position=(ha, 0),
                            )
                        ex = expp.tile([P, 2, 512], BF16, name="ex", tag=f"ex{p}")
                        nc.scalar.activation(
                            ex[:, :, j0:512], sc[:, :, j0:512],
                            mybir.ActivationFunctionType.Exp,
                            scale=inv64,
                        )
                        if c >= 4 * g:  # diagonal block: zero ks > qs
                            nc.vector.scalar_tensor_tensor(
                                ex[:, :, j0:j0 + P],
                                ex[:, :, j0:j0 + P],
                                1.0, tri2_sb,
                                MULT, MULT,
                            )
                        if dbg and g == 1 and c == 0:
                            dext = expp.tile([P, 2, 512], F32, name="dext",
                                             tag=f"dext{p}")
                            nc.vector.tensor_copy(dext, ex)
                            nc.sync.dma_start(dexp.ap()[p], dext)
                        if prev is not None:
                            av_den(*prev)
                        prev = (p, c, ex, j0)
                    if p == 0 and g < 3 and _il:
                        # interleave the next block's k/q projections + the v
                        # chunks it needs, at low priority so the tensor
                        # engine only runs them in ACT-bound gaps.
                        import contextlib
                        prio = (tc.high_priority(offset=-1_000_000)
                                if os.environ.get("KQV_PRIO", "1") == "1"
                                else contextlib.nullcontext())
                        with prio:
                            for half in range(2):
                                proj_block(qvp, "qv", wka_sb, wkb_sb,
                                           512 * (g + 1) + 256 * half, 256,
                                           ka_sb, kb_sb, rp2)
                            for half in range(2):
                                proj_block(qvp, "qv", wqa_sb, wqb_sb,
                                           512 * (g + 1) + 256 * half, 256,
                                           qa_sb, qb_sb, rp2)
                            for sc2 in range(4 * g + 4, 4 * g + 8, 2):
                                vchunk2(qvp, "qv", sc2)
                if prev is not None:
                    av_den(*prev)  # drain the deferred pipeline at block end
                # division: recip of the 4 denominator rows, broadcast to
                # 64-row bands, multiply outp into the projection rhs.
                recip = divp.tile([P, 512], F32, name="recip", tag="recip")
                nc.vector.reciprocal_approx_fast(
                    recip[0:97, :], den[0:97, :])
                if dbg:
                    ddent = divp.tile([P, 512], F32, name="ddent", tag="ddent")
                    nc.vector.tensor_copy(ddent, den)
                    nc.sync.dma_start(dden.ap()[g], ddent)
                    nc.sync.dma_start(drecip.ap()[g], recip)
                for t in range(2):
                    bc_ps = acc.tile([P, 512], F32, name=f"bcps{t}", tag="den")
                    nc.tensor.matmul(
                        bc_ps, sel_sb[:, t, :], recip[0:97, :],
                        start=True, stop=True,
                    )
                    bc = divp.tile([P, 512], F32, name=f"bc{t}", tag=f"bc{t}")
                    nc.vector.tensor_copy(bc, bc_ps)
                    if dbg:
                        doutpt = divp.tile([P, 512], F32, name="doutpt",
                                           tag="ddent")
                        nc.vector.tensor_copy(doutpt, outp[t])
                        nc.sync.dma_start(doutp.ap()[g, t], doutpt)
                        nc.sync.dma_start(dbc.ap()[g, t], bc)
                    nc.vector.scalar_tensor_tensor(
                        outT_sb[:, t, q0:q0 + 512],
                        outp[t], 1.0, bc,
                        MULT, MULT,
                    )

        if dbg:
            for i, t in enumerate((qa_sb, qb_sb, ka_sb, kb_sb)):
                dqt = const.tile([P, S], F32, name=f"dq{i}")
                nc.vector.tensor_copy(dqt, t)
                nc.sync.dma_start(dq.ap()[i], dqt)
            dvt = const.tile([P, NSC * NH * DK], F32, name="dvt")
            nc.vector.tensor_copy(
                dvt, v_sb.rearrange("p a b c -> p (a b c)"))
            nc.sync.dma_start(dv.ap(), dvt)
            doutt = const.tile([P, 2 * S], F32, name="doutt")
            nc.vector.tensor_copy(
                doutt, outT_sb.rearrange("p a s -> p (a s)"))
            nc.sync.dma_start(dout.ap(), doutt)

        # ---- phase 3: final projection (partial over this core's 256 dims) ----
        with tc.tile_pool(name="fps", bufs=6, space="PSUM") as fps_pool, \
             tc.tile_pool(name="fsb", bufs=6) as fsb_pool:
            for ec in range(D // P):
                for sb in range(NSB):
                    fps = fps_pool.tile([P, 512], F32, name="fps", tag="f")
                    for dc in range(2):
                        nc.tensor.matmul(
                            fps,
                            wot_sb[:, dc, P * ec:P * ec + P],
                            outT_sb[:, dc, 512 * sb:512 * sb + 512],
                            start=(dc == 0), stop=(dc == 1),
                        )
                    fsb = fsb_pool.tile([P, 512], BF16, name="fsb", tag="fo")
                    nc.any.tensor_copy(fsb, fps)
                    nc.sync.dma_start(
                        fT.ap()[P * ec:P * ec + P, 512 * sb:512 * sb + 512],
                        fsb,
                    )
        const.release()
    nc.compile()
    return nc


def _host_inputs(x, freqs_cos, freqs_sin, wq, wk, wv, wo):
    """Build the 8 per-core input maps (all host-side numpy)."""
    import ml_dtypes
    bf16 = ml_dtypes.bfloat16

    cosT = np.ascontiguousarray(freqs_cos.T).astype(np.float32)  # [32, S]
    sinT = np.ascontiguousarray(freqs_sin.T).astype(np.float32)
    cc = np.tile(cosT, (4, 1)).astype(bf16)
    ss = np.tile(sinT, (4, 1)).astype(bf16)
    # tri[p, j] = 1 if p <= j else 0 (keep ks <= qs on the diagonal block)
    tri = np.triu(np.ones((P, P), dtype=np.float32))
    tri2 = np.concatenate([tri, tri], axis=1).astype(bf16)
    # selector for the denominator broadcast: out tile t rows 64i..64i+64
    # pick up recip row 32*(2t+i).
    sel = np.zeros((97, 2 * P), dtype=np.float32)
    for t in range(2):
        for i in range(2):
            sel[32 * (2 * t + i), P * t + 64 * i: P * t + 64 * i + 64] = 1.0

    idxA = np.concatenate([64 * h + np.arange(0, 64, 2) for h in range(NH)])
    idxB = idxA + 1

    in_maps = []
    for core in range(8):
        b, g = core // 4, core % 4
        hs = slice(EG * g, EG * (g + 1))
        wq_g, wk_g = wq[hs], wk[hs]
        m = {
            "xT": np.ascontiguousarray(x[b].T).astype(bf16),
            "wqa": np.ascontiguousarray(wq_g[idxA].T).astype(bf16),
            "wqb": np.ascontiguousarray(wq_g[idxB].T).astype(bf16),
            "wka": np.ascontiguousarray(wk_g[idxA].T).astype(bf16),
            "wkb": np.ascontiguousarray(wk_g[idxB].T).astype(bf16),
            "wvt": np.ascontiguousarray(wv[hs].T).astype(bf16),
            "wot": np.ascontiguousarray(wo[:, hs].T).astype(bf16),
            "cc": cc, "ss": ss, "tri2": tri2, "sel": sel,
        }
        in_maps.append(m)
    return in_maps


def kernel(x, freqs_cos, freqs_sin, mask, wq, wk, wv, wo):
    global _NC_CACHE
    x = np.asarray(x, dtype=np.float32)
    freqs_cos = np.asarray(freqs_cos, dtype=np.float32)
    freqs_sin = np.asarray(freqs_sin, dtype=np.float32)
    wq = np.asarray(wq, dtype=np.float32)
    wk = np.asarray(wk, dtype=np.float32)
    wv = np.asarray(wv, dtype=np.float32)
    wo = np.asarray(wo, dtype=np.float32)

    if _NC_CACHE is None:
        _NC_CACHE = _build_nc()
    nc = _NC_CACHE

    in_maps = _host_inputs(x, freqs_cos, freqs_sin, wq, wk, wv, wo)
    trace = os.environ.get("BASS_KERNEL_TRACE", "0") == "1"
    res = bass_utils.run_bass_kernel_spmd(
        nc, in_maps, core_ids=list(range(8)), trace=trace,
    )
    if trace and res.exec_time_ns is not None:
        print(f"HW exec time: {res.exec_time_ns} ns")
        _tr = getattr(res, "instructions_and_trace", None)
        if _tr:
            print(f"trace: {_tr[1]}")

    out = np.zeros((B, S, D), dtype=np.float32)
    for core in range(8):
        b = core // 4
        out[b] += res.results[core]["fT"].T.astype(np.float32)
    return out


# revision 38
# speedup vs baseline: 2.3886x; 1.0532x over previous
"""Trainium2 Bass kernel for causal multi-head attention with RoPE.

Problem (hardcoded): B=2, S=2048, D=1024, H=16 heads, DK=64, double 1/sqrt(dk)
scaling, causal mask, RoPE (interleaved pairs).

Sharding over 8 cores: core c -> batch b=c//4, head-group g=c%4 (4 heads each).
Each core computes q/k/v projections for its heads from x[b], RoPE, causal
attention, and a partial output projection (its 256 columns of the contraction
with wo).  Host sums the 4 partials per batch.

Design notes (v2 — bf16 matmul operands, engine rebalance):
  - all matmul operands are bf16 (fp32 moving operands stream ~2x slower).
  - xT [D, S] host-transposed; q/k in "T layout" [e_local, S] with a global
    evens/odds row permutation: tile A = even rope components of all 4 heads
    (32 rows each), tile B = odds.  RoPE = 6 scalar_tensor_tensor per block.
  - scoresT[ks, qs] = k'^T q' with K=32 A/B accumulating pairs; heads at PE
    row bands 0/32/64/96 so 4 heads' matmuls run concurrently.
  - exp on the ACT engine (the true bottleneck: ~1 elem/lane/cycle @1.2GHz),
    psum->sbuf bf16, scale=1/64 fused.  Diagonal 128x128 blocks masked by a
    0/1 upper-tri multiply after exp (DVE, bf16 2x mode).
  - attn@v column-tiled: heads 2p/2p+1 at PE col bands (0,0)/(0,64), each
    streaming its own exp rhs -> out tile [128, 512] (dv rows packed 2 heads).
  - softmax denominators via 4 M=1 col-strip matmuls (ones lhsT) accumulating
    at psum partitions 0/32/64/96; reciprocal_approx_fast (DVE custom op);
    broadcast to 64-row bands with gpsimd partition_broadcast; one STT per
    out tile divides and writes the bf16 rhs of the final projection.
  - projections for q block g+1 and v chunks run interleaved with attention
    block g so the tensor engine fills ACT-bound gaps.
  - final projection at the end (psum->sbuf bf16 copies on any-engine), bf16
    DMA out; host sums the 4 partials per batch in fp32.
"""

import os
import numpy as np

import concourse.bass as bass
import concourse.bacc as bacc
import concourse.mybir as mybir
import concourse.tile as tile
from concourse import bass_utils

F32 = mybir.dt.float32
BF16 = mybir.dt.bfloat16
MULT = mybir.AluOpType.mult
ADD = mybir.AluOpType.add
SUB = mybir.AluOpType.subtract

B, S, D, H = 2, 2048, 1024, 16
DK = 64
NH = 4          # heads per core
EG = NH * DK    # 256 local e-dims per core
P = 128
NDC = D // P    # 8 d-chunks
NSC = S // P    # 16 s-chunks of 128
NSB = S // 512  # 4 s-blocks of 512

_NC_CACHE = None


def _build_nc():
    nc = bacc.Bacc("TRN2", target_bir_lowering=False, debug=False, num_devices=8)

    xT = nc.dram_tensor("xT", [D, S], BF16, kind="ExternalInput")
    wqa = nc.dram_tensor("wqa", [D, P], BF16, kind="ExternalInput")
    wqb = nc.dram_tensor("wqb", [D, P], BF16, kind="ExternalInput")
    wka = nc.dram_tensor("wka", [D, P], BF16, kind="ExternalInput")
    wkb = nc.dram_tensor("wkb", [D, P], BF16, kind="ExternalInput")
    wvt = nc.dram_tensor("wvt", [D, EG], BF16, kind="ExternalInput")
    wot = nc.dram_tensor("wot", [EG, D], BF16, kind="ExternalInput")
    cc = nc.dram_tensor("cc", [P, S], BF16, kind="ExternalInput")
    ss = nc.dram_tensor("ss", [P, S], BF16, kind="ExternalInput")
    tri2 = nc.dram_tensor("tri2", [P, 2 * P], BF16, kind="ExternalInput")
    sel = nc.dram_tensor("sel", [97, 2 * P], F32, kind="ExternalInput")
    fT = nc.dram_tensor("fT", [D, S], BF16, kind="ExternalOutput")
    dbg = os.environ.get("BASS_KERNEL_DEBUG", "0") == "1"
    if dbg:
        dq = nc.dram_tensor("dq", [4, P, S], F32, kind="ExternalOutput")
        dv = nc.dram_tensor("dv", [P, NSC * NH * DK], F32, kind="ExternalOutput")
        dden = nc.dram_tensor("dden", [NSB, P, 512], F32, kind="ExternalOutput")
        drecip = nc.dram_tensor("drecip", [NSB, P, 512], F32,
                                kind="ExternalOutput")
        dbc = nc.dram_tensor("dbc", [NSB, 2, P, 512], F32, kind="ExternalOutput")
        doutp = nc.dram_tensor("doutp", [NSB, 2, P, 512], F32,
                               kind="ExternalOutput")
        dexp = nc.dram_tensor("dexp", [2, P, 2, 512], F32, kind="ExternalOutput")
        dout = nc.dram_tensor("dout", [P, 2 * S], F32, kind="ExternalOutput")

    inv64 = 1.0 / 64.0

    with tile.TileContext(nc) as tc:
        const = tc.alloc_tile_pool(name="const", bufs=1)

        # ---- resident SBUF ----
        # DMA order tuned so the block-0 working set (k/q weights, cos/sin,
        # first 512 columns of xT) lands first and attention can start ~10us
        # in, while the rest streams behind.
        xT_sb = const.tile([P, NDC, S], BF16)

        def load_xT(cb):
            for dc in range(NDC):
                nc.sync.dma_start(
                    xT_sb[:, dc, 512 * cb:512 * cb + 512],
                    xT.ap()[P * dc:P * dc + P, 512 * cb:512 * cb + 512],
                )

        wka_sb = const.tile([P, NDC, P], BF16)
        nc.sync.dma_start(wka_sb, wka.ap().rearrange("(dc p) e -> p dc e", p=P))
        wkb_sb = const.tile([P, NDC, P], BF16)
        nc.sync.dma_start(wkb_sb, wkb.ap().rearrange("(dc p) e -> p dc e", p=P))
        load_xT(0)
        cc_sb = const.tile([P, S], BF16)
        nc.sync.dma_start(cc_sb, cc.ap())
        ss_sb = const.tile([P, S], BF16)
        nc.sync.dma_start(ss_sb, ss.ap())
        wqa_sb = const.tile([P, NDC, P], BF16)
        nc.sync.dma_start(wqa_sb, wqa.ap().rearrange("(dc p) e -> p dc e", p=P))
        wqb_sb = const.tile([P, NDC, P], BF16)
        nc.sync.dma_start(wqb_sb, wqb.ap().rearrange("(dc p) e -> p dc e", p=P))
        wvt_sb = const.tile([P, NDC, EG], BF16)
        nc.sync.dma_start(wvt_sb, wvt.ap().rearrange("(dc p) e -> p dc e", p=P))
        load_xT(1)
        tri2_sb = const.tile([P, 2, P], BF16)
        nc.sync.dma_start(tri2_sb, tri2.ap().rearrange("p (t c) -> p t c", t=2))
        sel_sb = const.tile([97, 2, P], F32)
        nc.sync.dma_start(sel_sb, sel.ap().rearrange("p (t c) -> p t c", t=2))
        load_xT(2)
        load_xT(3)
        wot_sb = const.tile([P, 2, D], BF16)
        nc.sync.dma_start(wot_sb, wot.ap().rearrange("(dc p) e -> p dc e", p=P))

        # per-512-block tiles for q/k/v so interleaved projection writes for
        # block g+1 never alias (whole-tile dep) attention reads of block g.
        qaB = [const.tile([P, 512], BF16, name=f"qaB{g}") for g in range(NSB)]
        qbB = [const.tile([P, 512], BF16, name=f"qbB{g}") for g in range(NSB)]
        kaB = [const.tile([P, 512], BF16, name=f"kaB{g}") for g in range(NSB)]
        kbB = [const.tile([P, 512], BF16, name=f"kbB{g}") for g in range(NSB)]
        vB = [const.tile([P, 4, NH, DK], BF16, name=f"vB{g}")
              for g in range(NSB)]
        outT_sb = const.tile([P, 2, S], BF16)
        ones_col = const.tile([P, 1], BF16)
        nc.vector.memset(ones_col, 1.0)

        def rope(pool, pc, col0, w, oa, ob, o0):
            """pc: [P, 2, w] bf16 sbuf (A slab 0, B slab 1) -> oa/ob[o0:o0+w].

            col0 is the GLOBAL column (for cos/sin); o0 the offset into the
            per-block output tiles."""
            sl = slice(col0, col0 + w)
            osl = slice(o0, o0 + w)
            t1 = pool.tile([P, 512], BF16, name="t1", tag="t1")
            t2 = pool.tile([P, 512], BF16, name="t2", tag="t2")
            nc.vector.scalar_tensor_tensor(
                t1[:, 0:w], pc[:, 0, :], 1.0, cc_sb[:, sl], MULT, MULT)
            nc.vector.scalar_tensor_tensor(
                t2[:, 0:w], pc[:, 1, :], 1.0, ss_sb[:, sl], MULT, MULT)
            nc.vector.scalar_tensor_tensor(
                oa[:, osl], t1[:, 0:w], 1.0, t2[:, 0:w], MULT, SUB)
            t3 = pool.tile([P, 512], BF16, name="t3", tag="t1")
            t4 = pool.tile([P, 512], BF16, name="t4", tag="t2")
            nc.vector.scalar_tensor_tensor(
                t3[:, 0:w], pc[:, 0, :], 1.0, ss_sb[:, sl], MULT, MULT)
            nc.vector.scalar_tensor_tensor(
                t4[:, 0:w], pc[:, 1, :], 1.0, cc_sb[:, sl], MULT, MULT)
            nc.vector.scalar_tensor_tensor(
                ob[:, osl], t3[:, 0:w], 1.0, t4[:, 0:w], MULT, ADD)

        def proj_block(pool, tag, wa, wb, col0, w, oa, ob, rope_pool,
                       o0=0, bufs=None):
            ps = pool.tile([P, 2, w], F32, name="ps", tag=tag, bufs=bufs)
            for i, wsel in enumerate((wa, wb)):
                for dc in range(NDC):
                    nc.tensor.matmul(
                        ps[:, i, :],
                        wsel[:, dc, :],
                        xT_sb[:, dc, col0:col0 + w],
                        start=(dc == 0), stop=(dc == NDC - 1),
                    )
            # one CAST frees the psum bank immediately; RoPE then runs on
            # bf16 SBUF operands at DVE 2x rate.
            pc = rope_pool.tile([P, 2, 512], BF16, name="pc", tag="pc")
            nc.vector.tensor_copy(pc[:, :, 0:w], ps)
            rope(rope_pool, pc[:, :, 0:w], col0, w, oa, ob, o0)

        def vchunk2(pool, tag, sc0, bufs=None):
            # two s-chunks through one [P, 2, 256] tile so every allocation
            # of the shared tag covers the full slot extent (range-aware
            # WAR tracking on psum slot reuse needs matching extents).
            pv = pool.tile([P, 2, EG], F32, name="pv2", tag=tag, bufs=bufs)
            for k in range(2):
                for dc in range(NDC):
                    nc.tensor.matmul(
                        pv[:, k, :],
                        xT_sb[:, dc, P * (sc0 + k):P * (sc0 + k) + P],
                        wvt_sb[:, dc, :],
                        start=(dc == 0), stop=(dc == NDC - 1),
                    )
            for k in range(2):
                sc = sc0 + k
                nc.vector.tensor_copy(
                    vB[sc // 4][:, sc % 4, :, :],
                    pv[:, k, :].rearrange("p (h e) -> p h e", h=NH))

        # ---- phase 1: k (all), q block 0, v chunks 0-3 ----
        with tc.tile_pool(name="p1", bufs=3, space="PSUM") as p1, \
             tc.tile_pool(name="rp1", bufs=2) as rp1:
            # KQV_MODE: full = interleave k/q/v projections with attention;
            # phased = everything upfront.
            _mode = os.environ.get("KQV_MODE", "full")
            _il = _mode == "full"
            proj_block(p1, "kq", wka_sb, wkb_sb, 0, 512, kaB[0], kbB[0], rp1)
            proj_block(p1, "kq", wqa_sb, wqb_sb, 0, 512, qaB[0], qbB[0], rp1)
            if not _il:
                for sb in range(1, NSB):
                    proj_block(p1, "kq", wka_sb, wkb_sb, 512 * sb, 512,
                               kaB[sb], kbB[sb], rp1)
                for sb in range(1, NSB):
                    proj_block(p1, "kq", wqa_sb, wqb_sb, 512 * sb, 512,
                               qaB[sb], qbB[sb], rp1)
            for sc0 in range(0, 4 if _il else NSC, 2):
                vchunk2(p1, "pv", sc0, bufs=2)

        # ---- phase 2: attention, interleaved with q/v projections ----
        with tc.tile_pool(name="scp", bufs=2, space="PSUM") as scp, \
             tc.tile_pool(name="acc", bufs=1, space="PSUM") as acc, \
             tc.tile_pool(name="qvp", bufs=1, space="PSUM") as qvp, \
             tc.tile_pool(name="expp", bufs=2) as expp, \
             tc.tile_pool(name="rp2", bufs=2) as rp2, \
             tc.tile_pool(name="divp", bufs=2) as divp:
            for g in range(NSB):
                q0 = 512 * g
                nclast = 4 * g + 3
                outp = [
                    acc.tile([P, 512], F32, name=f"outp{t}", tag=f"outp{t}")
                    for t in range(2)
                ]
                den = acc.tile([P, 512], F32, name="den", tag="den")
                # garbage rows must stay finite: recip + selector matmul read
                # the full 0:97 partition range.
                nc.vector.memset(den, 1.0)
                # Two passes of 2 heads each (sc pool double-buffers across
                # chunks), with attn@v/den EMISSION deferred one chunk: the
                # tensor queue then goes [scores(c), av(c-1), scores(c+1),
                # av(c), ...] so by the time av(c) reaches the queue head its
                # exp(c) is long done — no per-chunk PE drain, HAM stays warm.
                def av_den(p, c, ex, j0):
                    for i in range(2):
                        h = 2 * p + i
                        nc.tensor.matmul(
                            outp[p][64 * i:64 * i + 64, j0:512],
                            vB[c // 4][:, c % 4, h, :],
                            ex[:, i, j0:512],
                            start=(c == 0), stop=(c == nclast),
                            tile_position=(0, 64 * i),
                            skip_group_check=True,
                        )
                        nc.tensor.matmul(
                            den[32 * h:32 * h + 1, j0:512],
                            ones_col,
                            ex[:, i, j0:512],
                            start=(c == 0), stop=(c == nclast),
                            tile_position=(0, 32 * h),
                            skip_group_check=True,
                        )

                prev = None
                for p in range(2):
                    for c in range(nclast + 1):
                        j0 = max(0, P * (c - 4 * g))
                        sc = scp.tile([P, 2, 512], F32, name="sc", tag="sc")
                        for i in range(2):
                            h = 2 * p + i
                            ha = 32 * h
                            cb, cl = c // 4, P * (c % 4)
                            nc.tensor.matmul(
                                sc[:, i, j0:512],
                                kaB[cb][ha:ha + 32, cl:cl + P],
                                qaB[g][ha:ha + 32, j0:512],
                                start=True, stop=False,
                                tile_position=(ha, 0),
                            )
                            nc.tensor.matmul(
                                sc[:, i, j0:512],
                                kbB[cb][ha:ha + 32, cl:cl + P],
                                qbB[g][ha:ha + 32, j0:512],
                                start=False, stop=True,
                                tile_position=(ha, 0),
                            )
                        ex = expp.tile([P, 2, 512], BF16, name="ex", tag=f"ex{p}")
                        nc.scalar.activation(
                            ex[:, :, j0:512], sc[:, :, j0:512],
                            mybir.ActivationFunctionType.Exp,
                            scale=inv64,
                        )
                        if c >= 4 * g:  # diagonal block: zero ks > qs
                            nc.vector.scalar_tensor_tensor(
                                ex[:, :, j0:j0 + P],
                                ex[:, :, j0:j0 + P],
                                1.0, tri2_sb,
                                MULT, MULT,
                            )
                        if dbg and g == 1 and c == 0:
                            dext = expp.tile([P, 2, 512], F32, name="dext",
                                             tag=f"dext{p}")
                            nc.vector.tensor_copy(dext, ex)
                            nc.sync.dma_start(dexp.ap()[p], dext)
                        if prev is not None:
                            av_den(*prev)
                        prev = (p, c, ex, j0)
                    if p == 0 and g < 3 and _il:
                        # interleave the next block's k/q projections + the v
                        # chunks it needs, at low priority so the tensor
                        # engine only runs them in ACT-bound gaps.
                        import contextlib
                        prio = (tc.high_priority(offset=-1_000_000)
                                if os.environ.get("KQV_PRIO", "1") == "1"
                                else contextlib.nullcontext())
                        with prio:
                            for half in range(2):
                                proj_block(qvp, "qv", wka_sb, wkb_sb,
                                           512 * (g + 1) + 256 * half, 256,
                                           kaB[g + 1], kbB[g + 1], rp2,
                                           o0=256 * half)
                            for half in range(2):
                                proj_block(qvp, "qv", wqa_sb, wqb_sb,
                                           512 * (g + 1) + 256 * half, 256,
                                           qaB[g + 1], qbB[g + 1], rp2,
                                           o0=256 * half)
                            for sc2 in range(4 * g + 4, 4 * g + 8, 2):
                                vchunk2(qvp, "qv", sc2)
                if prev is not None:
                    av_den(*prev)  # drain the deferred pipeline at block end
                # division: recip of the 4 denominator rows, broadcast to
                # 64-row bands, multiply outp into the projection rhs.
                recip = divp.tile([P, 512], F32, name="recip", tag="recip")
                nc.vector.reciprocal_approx_fast(
                    recip[0:97, :], den[0:97, :])
                if dbg:
                    ddent = divp.tile([P, 512], F32, name="ddent", tag="ddent")
                    nc.vector.tensor_copy(ddent, den)
                    nc.sync.dma_start(dden.ap()[g], ddent)
                    nc.sync.dma_start(drecip.ap()[g], recip)
                for t in range(2):
                    bc_ps = acc.tile([P, 512], F32, name=f"bcps{t}", tag="den")
                    nc.tensor.matmul(
                        bc_ps, sel_sb[:, t, :], recip[0:97, :],
                        start=True, stop=True,
                    )
                    bc = divp.tile([P, 512], F32, name=f"bc{t}", tag=f"bc{t}")
                    nc.vector.tensor_copy(bc, bc_ps)
                    if dbg:
                        doutpt = divp.tile([P, 512], F32, name="doutpt",
                                           tag="ddent")
                        nc.vector.tensor_copy(doutpt, outp[t])
                        nc.sync.dma_start(doutp.ap()[g, t], doutpt)
                        nc.sync.dma_start(dbc.ap()[g, t], bc)
                    nc.vector.scalar_tensor_tensor(
                        outT_sb[:, t, q0:q0 + 512],
                        outp[t], 1.0, bc,
                        MULT, MULT,
                    )

        if dbg:
            for i, blocks in enumerate((qaB, qbB, kaB, kbB)):
                dqt = const.tile([P, S], F32, name=f"dq{i}")
                for bb in range(NSB):
                    nc.vector.tensor_copy(
                        dqt[:, 512 * bb:512 * bb + 512], blocks[bb])
                nc.sync.dma_start(dq.ap()[i], dqt)
            dvt = const.tile([P, NSC * NH * DK], F32, name="dvt")
            for bb in range(NSB):
                nc.vector.tensor_copy(
                    dvt[:, 1024 * bb:1024 * bb + 1024],
                    vB[bb].rearrange("p a b c -> p (a b c)"))
            nc.sync.dma_start(dv.ap(), dvt)
            doutt = const.tile([P, 2 * S], F32, name="doutt")
            nc.vector.tensor_copy(
                doutt, outT_sb.rearrange("p a s -> p (a s)"))
            nc.sync.dma_start(dout.ap(), doutt)

        # ---- phase 3: final projection (partial over this core's 256 dims) ----
        with tc.tile_pool(name="fps", bufs=6, space="PSUM") as fps_pool, \
             tc.tile_pool(name="fsb", bufs=6) as fsb_pool:
            for ec in range(D // P):
                for sb in range(NSB):
                    fps = fps_pool.tile([P, 512], F32, name="fps", tag="f")
                    for dc in range(2):
                        nc.tensor.matmul(
                            fps,
                            wot_sb[:, dc, P * ec:P * ec + P],
                            outT_sb[:, dc, 512 * sb:512 * sb + 512],
                            start=(dc == 0), stop=(dc == 1),
                        )
                    fsb = fsb_pool.tile([P, 512], BF16, name="fsb", tag="fo")
                    nc.any.tensor_copy(fsb, fps)
                    nc.sync.dma_start(
                        fT.ap()[P * ec:P * ec + P, 512 * sb:512 * sb + 512],
                        fsb,
                    )
        const.release()
    nc.compile()
    return nc


def _host_inputs(x, freqs_cos, freqs_sin, wq, wk, wv, wo):
    """Build the 8 per-core input maps (all host-side numpy)."""
    import ml_dtypes
    bf16 = ml_dtypes.bfloat16

    cosT = np.ascontiguousarray(freqs_cos.T).astype(np.float32)  # [32, S]
    sinT = np.ascontiguousarray(freqs_sin.T).astype(np.float32)
    cc = np.tile(cosT, (4, 1)).astype(bf16)
    ss = np.tile(sinT, (4, 1)).astype(bf16)
    # tri[p, j] = 1 if p <= j else 0 (keep ks <= qs on the diagonal block)
    tri = np.triu(np.ones((P, P), dtype=np.float32))
    tri2 = np.concatenate([tri, tri], axis=1).astype(bf16)
    # selector for the denominator broadcast: out tile t rows 64i..64i+64
    # pick up recip row 32*(2t+i).
    sel = np.zeros((97, 2 * P), dtype=np.float32)
    for t in range(2):
        for i in range(2):
            sel[32 * (2 * t + i), P * t + 64 * i: P * t + 64 * i + 64] = 1.0

    idxA = np.concatenate([64 * h + np.arange(0, 64, 2) for h in range(NH)])
    idxB = idxA + 1

    in_maps = []
    for core in range(8):
        b, g = core // 4, core % 4
        hs = slice(EG * g, EG * (g + 1))
        wq_g, wk_g = wq[hs], wk[hs]
        m = {
            "xT": np.ascontiguousarray(x[b].T).astype(bf16),
            "wqa": np.ascontiguousarray(wq_g[idxA].T).astype(bf16),
            "wqb": np.ascontiguousarray(wq_g[idxB].T).astype(bf16),
            "wka": np.ascontiguousarray(wk_g[idxA].T).astype(bf16),
            "wkb": np.ascontiguousarray(wk_g[idxB].T).astype(bf16),
            "wvt": np.ascontiguousarray(wv[hs].T).astype(bf16),
            "wot": np.ascontiguousarray(wo[:, hs].T).astype(bf16),
            "cc": cc, "ss": ss, "tri2": tri2, "sel": sel,
        }
        in_maps.append(m)
    return in_maps


def kernel(x, freqs_cos, freqs_sin, mask, wq, wk, wv, wo):
    global _NC_CACHE
    x = np.asarray(x, dtype=np.float32)
    freqs_cos = np.asarray(freqs_cos, dtype=np.float32)
    freqs_sin = np.asarray(freqs_sin, dtype=np.float32)
    wq = np.asarray(wq, dtype=np.float32)
    wk = np.asarray(wk, dtype=np.float32)
    wv = np.asarray(wv, dtype=np.float32)
    wo = np.asarray(wo, dtype=np.float32)

    if _NC_CACHE is None:
        _NC_CACHE = _build_nc()
    nc = _NC_CACHE

    in_maps = _host_inputs(x, freqs_cos, freqs_sin, wq, wk, wv, wo)
    trace = os.environ.get("BASS_KERNEL_TRACE", "0") == "1"
    res = bass_utils.run_bass_kernel_spmd(
        nc, in_maps, core_ids=list(range(8)), trace=trace,
    )
    if trace and res.exec_time_ns is not None:
        print(f"HW exec time: {res.exec_time_ns} ns")
        _tr = getattr(res, "instructions_and_trace", None)
        if _tr:
            print(f"trace: {_tr[1]}")

    out = np.zeros((B, S, D), dtype=np.float32)
    for core in range(8):
        b = core // 4
        out[b] += res.results[core]["fT"].T.astype(np.float32)
    return out
